# revision 27
# baseline (speedup 1.0000x reference)
"""CARAFE + MSGConv Trainium2 kernel (8 NeuronCores, spatial x batch sharding).

out[c, i, j] = sum_{p,q} W[5p+q, i, j] * Xpad[c, i//2 + p - 2, j//2 + q - 2]
 (CARAFE taps live at source resolution; identical for both subpixel parities).

Per core: one batch element (core//4) and a 16-source-row block (core%4).

v2 changes vs baseline:
 - dw tap loops split Vector (STT) || TensorEngine (diagonal-weight matmuls
   accumulating in PSUM), combined with one tensor_add then Silu.
 - b4 transpose moved from PE (transpose + PSUM->SBUF copy) to DMA xbar
   transpose (SBUF->SBUF, [128,128] bf16 blocks).
 - repl matmuls merged 2 row-pairs per call (16x N=200).
 - out matmuls write 4 jb blocks into one [128,512] PSUM bank; single
   staging copy per row-pair.
 - wcat softmax scaling on vector tensor_scalar (2x mode).
 - input DMA ordering: front-critical tensors first, xt/back-end consts later.
"""

import sys

sys.path.insert(0, "/opt/trn_rl_repo")

from contextlib import ExitStack

import ml_dtypes
import numpy as np

import concourse.bass as bass
import concourse.tile as tile
from concourse import bacc, library_config, mybir
from concourse.bass_utils import run_bass_kernel_spmd

BF16 = mybir.dt.bfloat16
F32 = mybir.dt.float32
I16 = mybir.dt.int16
AF = mybir.ActivationFunctionType
OP = mybir.AluOpType
nbf = ml_dtypes.bfloat16

C = 128
H = W = 64
NCORES = 8
XR = 24          # X shard rows (16 + 4 halo each side)
XW = 68          # padded width for dw slabs only
NEG = -30.0      # additive pre-activation mask; SiLU(-30) ~= -2.8e-12

# dw tap split: taps [0, NPE) on the TensorEngine, [NPE, 25) on Vector
NPE1 = 16        # dw1 PE taps
NPE2 = 16        # dw2 PE taps


# ======================================================================
# host-side parameter prep
# ======================================================================

def _fold_1x1(w, s):
    return (w[:, :, 0, 0] * s[:, None]).T.copy()


def _dw_taps(w, s, k):
    ch = w.shape[0]
    out = np.zeros((ch, 25), np.float32)
    off = (5 - k) // 2
    for ty in range(k):
        for tx in range(k):
            out[:, 5 * (ty + off) + (tx + off)] = w[:, 0, ty, tx] * s
    return out


def _host_consts(inputs):
    d = {}
    w_cv1 = _fold_1x1(inputs["comp_cv1_w"], inputs["comp_cv1_s"])
    b_cv1 = inputs["comp_cv1_b"].reshape(32, 1)
    w3 = _dw_taps(inputs["comp_dw3_w"], inputs["comp_dw3_s"], 3)
    w5 = _dw_taps(inputs["comp_dw5_w"], inputs["comp_dw5_s"], 5)
    w_dwp = np.tile(np.concatenate([w3, w5], 0), (4, 1))
    b_dwp = np.tile(
        np.concatenate([inputs["comp_dw3_b"], inputs["comp_dw5_b"]]), 4
    ).reshape(128, 1)
    w_px = _fold_1x1(inputs["comp_px_w"], inputs["comp_px_s"])
    b_px = inputs["comp_px_b"].reshape(64, 1)
    we = _fold_1x1(inputs["enc_cv1_w"], inputs["enc_cv1_s"])
    w_ecv1 = np.concatenate([we, np.ones((1, 50), np.float32)], 0)
    b_ecv1 = inputs["enc_cv1_b"].reshape(50, 1)
    e3 = _dw_taps(inputs["enc_dw3_w"], inputs["enc_dw3_s"], 3)
    e5 = _dw_taps(inputs["enc_dw5_w"], inputs["enc_dw5_s"], 5)
    w_edwp = np.tile(np.concatenate([e3, e5], 0), (2, 1))
    b_edwp = np.tile(
        np.concatenate([inputs["enc_dw3_b"], inputs["enc_dw5_b"]]), 2
    ).reshape(100, 1)
    wpx = _fold_1x1(inputs["enc_px_w"], inputs["enc_px_s"])
    w_epx = np.concatenate([wpx, inputs["enc_px_b"].reshape(1, 100)], 0)

    # packA bf16 [128, 510]: w_cv1 | w_px | w_ecv1 | w_epx_a | px2 masked
    # halves (even/odd dw1 groups) | epx_b masked halves (lo/hi e2p group)
    pa = np.zeros((128, 574), np.float32)
    pa[0:128, 0:32] = w_cv1
    pa[0:64, 32:96] = w_px
    pa[0:65, 96:146] = w_ecv1
    pa[0:50, 146:246] = w_epx[0:50]
    pa[50:51, 146:246] = w_epx[100:101]
    pa[0:32, 246:310] = w_px[32:64]      # g even (lhsT half [0:64] base 0/64)
    pa[64:96, 246:310] = w_px[32:64]
    pa[32:64, 310:374] = w_px[32:64]     # g odd
    pa[96:128, 310:374] = w_px[32:64]
    pa[0:50, 374:474] = w_epx[50:100]    # e2p group 0 (t < 4)
    pa[50:100, 474:574] = w_epx[50:100]  # e2p group 1 (t >= 4)
    d["packa"] = pa.astype(nbf)
    # packB f32 [128, 55]
    pb = np.zeros((128, 55), np.float32)
    pb[:, 0:25] = w_dwp
    pb[:, 25:26] = b_dwp
    pb[0:100, 26:51] = w_edwp
    pb[0:100, 51:52] = b_edwp
    pb[0:32, 52:53] = b_cv1
    pb[0:64, 53:54] = b_px
    pb[0:50, 54:55] = b_ecv1
    d["packb"] = pb

    # diagonal lhsT for the PE dw taps
    dg1 = np.zeros((128, NPE1 * 128), np.float32)
    for t in range(NPE1):
        dg1[np.arange(128), 128 * t + np.arange(128)] = w_dwp[:, t]
    d["diag1"] = dg1.astype(nbf)
    dg2 = np.zeros((100, NPE2 * 100), np.float32)
    for t in range(NPE2):
        dg2[np.arange(100), 100 * t + np.arange(100)] = w_edwp[:, t]
    d["diag2"] = dg2.astype(nbf)

    d["ones1"] = np.ones((1, 32), nbf)

    # replS [128, 4*128]: lhsT blocks per (jb, s); output pixel partition
    # within a jb block is s-major: m = 16*yl + xl, placed at psum
    # partitions 32s+m by the matmul's out slice.
    rp = np.zeros((128, 512), np.float32)
    for jb in range(4):
        for s_ in range(4):
            for yl in range(2):
                for xl in range(16):
                    rp[64 * yl + 16 * jb + xl,
                       128 * jb + 32 * s_ + 16 * yl + xl] = 1.0
    d["repl"] = rp.astype(nbf)

    # sidx [128, 200] int16, shared by all four t-pair scatter calls:
    # partition = s-major pixel-in-block (32s + 16yl + xl), data col
    # (th, jb, k) -> target slot 512*th + 128*jb + 20u + v (uv pitch 20,
    # cols 120..127 of each 128 block are pad for the xbar transpose).
    si = np.full((128, 200), -1, np.int16)
    for part in range(128):
        s_, m = divmod(part, 32)
        yl, xl = divmod(m, 16)
        for th in range(2):
            for jb in range(4):
                for k in range(25):
                    p, q = divmod(k, 5)
                    if not (0 <= 16 * jb + xl + q - 2 < 64):
                        continue
                    si[part, 100 * th + 25 * jb + k] = (
                        512 * th + 128 * jb + 20 * (yl + p) + (xl + q)
                    )
    d["sidx"] = si
    return d


def _host_shard(X, core):
    b, ri = divmod(core, 4)
    r0 = 16 * ri - 4
    xs = np.zeros((C, XR, W), np.float32)
    lo, hi = max(0, r0), min(H, r0 + XR)
    xs[:, lo - r0 : hi - r0, :] = X[b, :, lo:hi, :]
    mrow = np.zeros((1, XR, W), np.float32)
    for r in range(XR):
        if not (0 <= r0 + r < H):
            mrow[0, r, :] = NEG
    emask = np.zeros((1, 20, W), np.float32)
    for r in range(20):
        if not (0 <= (16 * ri - 2) + r < H):
            emask[0, r, :] = NEG
    xsb = xs.astype(nbf)
    # pre-transposed X slabs, one [120, 128] per block (column-padded)
    xsp = np.zeros((C, XR, XW), nbf)
    xsp[:, :, 2 : 2 + W] = xsb
    xt = np.zeros((120, 32 * 128), nbf)
    for B in range(32):
        t, jb = divmod(B, 4)
        slab = xsp[:, 2 * t + 2 : 2 * t + 8, 16 * jb : 16 * jb + 20]
        xt[:, 128 * B : 128 * B + 128] = slab.reshape(C, 120).T
    return (
        xsb.reshape(C, XR * W),
        mrow.reshape(1, XR * W).astype(nbf),
        emask.reshape(1, 20 * W).astype(nbf),
        xt,
    )


# ======================================================================
# device kernel
# ======================================================================

def build_kernel():
    nc = bacc.Bacc(
        "TRN2",
        target_bir_lowering=False,
        debug=False,
        enable_asserts=False,
        num_devices=NCORES,
    )

    def din(name, shape, dt):
        return nc.dram_tensor(name, list(shape), dt, kind="ExternalInput").ap()

    x_d = din("x", (128, XR * W), BF16)
    xt_d = din("xt", (120, 32 * 128), BF16)
    mrow_d = din("mrow", (1, XR * W), BF16)
    emask_d = din("emask", (1, 20 * W), BF16)
    ones1_d = din("ones1", (1, 32), BF16)
    packa_d = din("packa", (128, 574), BF16)
    packb_d = din("packb", (128, 55), F32)
    diag1_d = din("diag1", (128, NPE1 * 128), BF16)
    diag2_d = din("diag2", (100, NPE2 * 100), BF16)
    repl_d = din("repl", (128, 512), BF16)
    sidx_d = din("sidx", (128, 200), I16)
    out_d = nc.dram_tensor("out", [128, 32 * 128], F32, kind="ExternalOutput").ap()
    out3 = out_d.rearrange("c (r j) -> c r j", j=128)

    with tile.TileContext(nc) as tc, ExitStack() as ctx:
        cpool = ctx.enter_context(tc.tile_pool(name="consts", bufs=1))
        work = ctx.enter_context(tc.tile_pool(name="work", bufs=1))
        psB = ctx.enter_context(tc.tile_pool(name="psB", bufs=2, space="PSUM"))
        spool = ctx.enter_context(tc.tile_pool(name="stage", bufs=3))
        bpool = ctx.enter_context(tc.tile_pool(name="b4s", bufs=6))
        psA_cm = tc.tile_pool(name="psA", bufs=2, space="PSUM")
        psA = psA_cm.__enter__()
        psD_cm = tc.tile_pool(name="psD", bufs=1, space="PSUM")
        psD = psD_cm.__enter__()

        nc.gpsimd.load_library(library_config.local_scatter)

        def cload(ap_d, shape, dt, eng=None):
            t = cpool.tile(list(shape), dt, tag=ap_d.tensor.name)
            (eng or nc.sync).dma_start(t[:], ap_d)
            return t

        # front-critical loads first, in queue-program order per engine
        xb = cpool.tile([128, XR * W], BF16, tag="x")
        for ch, eng in enumerate((nc.sync, nc.scalar, nc.gpsimd)):
            eng.dma_start(
                xb[:, 8 * W * ch : 8 * W * (ch + 1)],
                x_d[:, 8 * W * ch : 8 * W * (ch + 1)],
            )
        packa = cload(packa_d, (128, 574), BF16)
        mrow = cload(mrow_d, (1, XR * W), BF16, eng=nc.scalar)
        packb = cload(packb_d, (128, 55), F32, eng=nc.scalar)
        ones1 = cload(ones1_d, (1, 32), BF16, eng=nc.scalar)
        diag1 = cload(diag1_d, (128, NPE1 * 128), BF16)
        diag2 = cload(diag2_d, (100, NPE2 * 100), BF16)
        # back-end consts (needed later) on the gpsimd software queue
        repl = cload(repl_d, (128, 512), BF16, eng=nc.gpsimd)
        sidx = cload(sidx_d, (128, 200), I16, eng=nc.gpsimd)
        xt = cpool.tile([120, 32 * 128], BF16, tag="xt")

        w_cv1 = packa[0:128, 0:32]
        w_px = packa[0:64, 32:96]
        w_ecv1 = packa[0:65, 96:146]
        w_epx = packa[0:101, 146:246]
        w_dwp = packb[0:128, 0:25]
        b_dwp = packb[0:128, 25:26]
        w_edwp = packb[0:100, 26:51]
        b_edwp = packb[0:100, 51:52]
        b_cv1 = packb[0:32, 52:53]
        b_px = packb[0:64, 53:54]
        b_ecv1 = packb[0:50, 54:55]

        # warmup: trigger the local_scatter ucode library load early so it
        # overlaps the conv front instead of stalling the first real scatter
        warm = work.tile([16, 16], BF16)
        nc.gpsimd.local_scatter(
            warm[:], packa[0:16, 0:2], sidx[:][0:16, 0:2],
            channels=16, num_elems=16, num_idxs=2,
        )

        # persistent working tensors (all 64-wide / contiguous)
        x12 = work.tile([64, XR * W], BF16)        # x1 (0:32) + x2 (32:64)
        enc_in = work.tile([65, 20 * W], BF16)     # px out + mask row
        e1c = work.tile([51, 20 * W], BF16)        # enc cv1 out + ones row
        x1p = work.tile([128, 9 * XW + 8], BF16)   # packed x1 (68-pitch)
        e1p = work.tile([100, 12 * XW + 8], BF16)  # packed enc x1 (68-pitch)
        ET = work.tile([128, 800], BF16)
        expv = work.tile([128, 800], BF16)         # [s][t][k]
        S = work.tile([128, 32], F32)
        R = work.tile([128, 32], F32)
        wcat = work.tile([128, 800], BF16)         # [t][s][k]
        dall = work.tile([128, 3200], BF16)
        b4t = work.tile([128, 8 * 512], BF16)      # per t: [4jb x 128]

        xb3 = xb[:].rearrange("p (r c) -> p r c", c=W)
        x12_3 = x12[:].rearrange("p (r c) -> p r c", c=W)
        e1c3 = e1c[:].rearrange("p (r c) -> p r c", c=W)
        x1p3 = x1p[:, 0 : 9 * XW].rearrange("p (r c) -> p r c", c=XW)
        e1p3 = e1p[:, 0 : 12 * XW].rearrange("p (r c) -> p r c", c=XW)
        ET3 = ET[:].rearrange("p (t e) -> p t e", e=100)
        exp3 = expv[:].rearrange("p (s t k) -> p s t k", s=4, t=8)

        # zero only the dw-slab pad columns (cols 0:2 and 66:68)
        nc.vector.memset(x1p[:, 9 * XW : 9 * XW + 8], 0.0)
        nc.vector.memset(e1p[:, 12 * XW : 12 * XW + 8], 0.0)
        nc.vector.memset(x1p3[:, :, 0:2], 0.0)
        nc.vector.memset(x1p3[:, :, 66:68], 0.0)
        nc.vector.memset(e1p3[:, :, 0:2], 0.0)
        nc.vector.memset(e1p3[:, :, 66:68], 0.0)
        nc.vector.memset(e1c[:], 1.0)
        nc.sync.dma_start(enc_in[64:65, :], emask_d)

        # ---- comp cv1: 1x1 conv 128->32 (+ SiLU + out-of-image row mask)
        for ch in range(3):
            ps = psA.tile([32, 512], F32, tag="convps")
            nc.tensor.matmul(
                ps[:], w_cv1, xb[:, 512 * ch : 512 * (ch + 1)],
                start=True, stop=False,
            )
            nc.tensor.matmul(
                ps[:], ones1[:], mrow[:, 512 * ch : 512 * (ch + 1)],
                start=False, stop=True,
            )
            nc.scalar.activation(
                x12[0:32, 512 * ch : 512 * (ch + 1)], ps[:],
                AF.Silu, bias=b_cv1,
            )

        # ---- comp dw3/dw5 (unified 5x5 taps, rows packed 4x32)
        # taps [0, NPE1) on the TensorEngine (diag lhsT, PSUM accumulate),
        # taps [NPE1, 25) on Vector (STT chain); combined + Silu.
        for g, eng in enumerate((nc.sync, nc.scalar, nc.gpsimd, nc.sync)):
            eng.dma_start(
                x1p3[32 * g : 32 * g + 32, 0:9, 2 : 2 + W],
                x12_3[0:32, 5 * g : 5 * g + 9, :],
            )
        FS = 5 * XW                    # 340
        ps1 = psD.tile([128, FS], F32, tag="dw1")
        for t in range(NPE1):
            ty, tx = divmod(t, 5)
            nc.tensor.matmul(
                ps1[:], diag1[:, 128 * t : 128 * t + 128],
                x1p[:, ty * XW + tx : ty * XW + tx + FS],
                start=(t == 0), stop=(t == NPE1 - 1),
            )
        acc_a = work.tile([128, FS], BF16)
        av = acc_a[:]
        for i, t in enumerate(range(NPE1, 25)):
            ty, tx = divmod(t, 5)
            sv = x1p[:, ty * XW + tx : ty * XW + tx + FS]
            if i == 0:
                nc.vector.tensor_scalar(av, sv, w_dwp[:, t : t + 1], None, OP.mult)
            else:
                nc.vector.scalar_tensor_tensor(
                    av, sv, w_dwp[:, t : t + 1], av, OP.mult, OP.add
                )
        nc.vector.tensor_add(av, av, ps1[:])
        x2q = work.tile([128, 5 * W], BF16)
        nc.scalar.activation(
            x2q[:].rearrange("p (r c) -> p r c", c=W),
            acc_a[:].rearrange("p (r c) -> p r c", c=XW)[:, 0:5, 0:W],
            AF.Silu, bias=b_dwp,
        )

        # ---- comp px: 1x1 conv 64->64 (+ SiLU), split K: x1 from x12 rows,
        # x2 read directly from the packed x2p slabs (no writeback DMA)
        w_px1 = packa[0:32, 32:96]
        for g in range(4):
            ps = psA.tile([64, 5 * W], F32, tag="convps")
            nc.tensor.matmul(
                ps[:], w_px1,
                x12[0:32, (2 + 5 * g) * W : (7 + 5 * g) * W],
                start=True, stop=False,
            )
            base, cols = 64 * (g // 2), (246 if g % 2 == 0 else 310)
            nc.tensor.matmul(
                ps[:], packa[base : base + 64, cols : cols + 64],
                x2q[base : base + 64, :],
                start=False, stop=True,
            )
            nc.scalar.activation(
                enc_in[0:64, 5 * g * W : (5 * g + 5) * W], ps[:],
                AF.Silu, bias=b_px,
            )

        # ---- enc cv1: 1x1 conv 64->50 (+ SiLU, mask row rides K=65);
        # e1p pack DMAs issued as soon as their source rows are done
        for r0, nr in ((0, 8), (8, 8), (16, 4)):
            ps = psA.tile([50, 512], F32, tag="convps")
            nc.tensor.matmul(
                ps[:, : nr * W], w_ecv1,
                enc_in[0:65, r0 * W : (r0 + nr) * W],
                start=True, stop=True,
            )
            nc.scalar.activation(
                e1c[0:50, r0 * W : (r0 + nr) * W], ps[:, : nr * W],
                AF.Silu, bias=b_ecv1,
            )
            if r0 == 8:
                nc.sync.dma_start(
                    e1p3[0:50, 0:12, 2 : 2 + W], e1c3[0:50, 0:12, :]
                )
        nc.scalar.dma_start(
            e1p3[50:100, 0:12, 2 : 2 + W], e1c3[0:50, 8:20, :]
        )
        FS2 = 8 * XW                   # 544
        HF = FS2 // 2                  # 272
        ps2a = psD.tile([100, HF], F32, tag="dw2a")
        ps2b = psD.tile([100, HF], F32, tag="dw2b")
        for t in range(NPE2):
            ty, tx = divmod(t, 5)
            base = ty * XW + tx
            nc.tensor.matmul(
                ps2a[:], diag2[:, 100 * t : 100 * t + 100],
                e1p[0:100, base : base + HF],
                start=(t == 0), stop=(t == NPE2 - 1),
            )
        for t in range(NPE2):
            ty, tx = divmod(t, 5)
            base = ty * XW + tx
            nc.tensor.matmul(
                ps2b[:], diag2[:, 100 * t : 100 * t + 100],
                e1p[0:100, base + HF : base + FS2],
                start=(t == 0), stop=(t == NPE2 - 1),
            )
        acc2_a = work.tile([100, FS2], BF16)
        av2 = acc2_a[:]
        for i, t in enumerate(range(NPE2, 25)):
            ty, tx = divmod(t, 5)
            sv = e1p[:, ty * XW + tx : ty * XW + tx + FS2]
            if i == 0:
                nc.vector.tensor_scalar(av2, sv, w_edwp[:, t : t + 1], None, OP.mult)
            else:
                nc.vector.scalar_tensor_tensor(
                    av2, sv, w_edwp[:, t : t + 1], av2, OP.mult, OP.add
                )
        nc.vector.tensor_add(acc2_a[:, 0:HF], acc2_a[:, 0:HF], ps2a[:])
        nc.vector.tensor_add(acc2_a[:, HF:FS2], acc2_a[:, HF:FS2], ps2b[:])
        e2q = work.tile([100, 8 * W], BF16)
        nc.scalar.activation(
            e2q[:].rearrange("p (r c) -> p r c", c=W),
            acc2_a[:].rearrange("p (r c) -> p r c", c=XW)[:, 0:8, 0:W],
            AF.Silu, bias=b_edwp,
        )

        # xt load (needed by the out matmuls from ~mid-kernel only;
        # emitting it here avoids false semaphore deps in the conv front)
        for ch, eng in enumerate((nc.sync, nc.scalar)):
            eng.dma_start(
                xt[:, 2048 * ch : 2048 * (ch + 1)],
                xt_d[:, 2048 * ch : 2048 * (ch + 1)],
            )

        # ---- enc px (transposed output: M = 128 pixels per row-pair),
        # split K: e1 + bias row from e1c, e2 direct from e2p slabs
        w_epx_a = packa[0:51, 146:246]
        for t in range(8):
            g, lr = divmod(2 * t, 8)
            ps = psA.tile([128, 100], F32, tag="convps")
            nc.tensor.matmul(
                ps[:], e1c[0:51, (2 + 2 * t) * W : (4 + 2 * t) * W],
                w_epx_a, start=True, stop=False,
            )
            cols = 374 if g == 0 else 474
            nc.tensor.matmul(
                ps[:], e2q[0:100, lr * W : (lr + 2) * W],
                packa[0:100, cols : cols + 100],
                start=False, stop=True,
            )
            nc.scalar.activation(ET[:, 100 * t : 100 * t + 100], ps[:], AF.Silu)

        # ---- softmax over 25 taps per subposition (no max-subtraction)
        for s in range(4):
            nc.scalar.activation(exp3[:, s], ET3[:, :, s::4], AF.Exp)
            nc.vector.tensor_reduce(
                S[:, 8 * s : 8 * s + 8], exp3[:, s], mybir.AxisListType.X, OP.add
            )
        nc.vector.reciprocal(R[:], S[:])
        psD_cm.__exit__(None, None, None)
        psA_cm.__exit__(None, None, None)
        psO = ctx.enter_context(tc.tile_pool(name="psO", bufs=3, space="PSUM"))

        # normalized weights, s-major: wcat[p, 200s + 25t + k]
        # = exp3[p, s, t, k] * R[p, 8s+t]
        R3 = R[:].rearrange("p (s u) -> p s u", s=4)
        wcat4 = wcat[:].rearrange("p (s t k) -> p s t k", s=4, t=8)
        for t in range(8):
            nc.vector.tensor_tensor(
                wcat4[:, :, t],
                exp3[:, :, t],
                R3[:, :, t : t + 1].to_broadcast((128, 4, 25)),
                OP.mult,
            )

        # repl matmuls: per (jb, s) one [32, 200] output at psum partition
        # offset 32s (s-major pixel packing); then cast into dall2 with the
        # (tp, th, jb, k) column interleave the t-pair scatters consume.
        dall2v = dall[:, 0:800].rearrange(
            "p (tp th j k) -> p tp th j k", tp=4, th=2, j=4
        )
        for jb in range(4):
            ps = psB.tile([128, 200], F32, tag="repl")
            for s_ in range(4):
                nc.tensor.matmul(
                    ps[32 * s_ : 32 * s_ + 32, :],
                    repl[:, 128 * jb + 32 * s_ : 128 * jb + 32 * s_ + 32],
                    wcat[:, 200 * s_ : 200 * s_ + 200],
                    start=True, stop=True,
                    tile_position=(0, 32 * s_),
                )
            src3 = ps[:].rearrange("p (tp th k) -> p tp th k", tp=4, th=2)
            if jb % 2 == 0:
                nc.vector.tensor_copy(dall2v[:, :, :, jb], src3)
            else:
                nc.scalar.copy(dall2v[:, :, :, jb], src3)

        # scatters first (4 s-compacted calls, one t-pair each: 200 idx,
        # 1024 out) so the gpsimd queue never stalls behind downstream
        # DMAs; then per t-pair: one chunked DMA transpose -> per row-pair
        # 4 matmuls into a [128,512] PSUM bank -> 1 straight staging copy
        # (s-major pixel columns; the host unpermutes) -> out DMA.
        for tp in range(4):
            nc.gpsimd.local_scatter(
                b4t[:, 1024 * tp : 1024 * tp + 1024],
                dall[:, 200 * tp : 200 * tp + 200],
                sidx[:],
                channels=128, num_elems=1024, num_idxs=200,
            )
        stgs = []
        for tp in range(4):
            b4 = bpool.tile([128, 8, 128], BF16, tag="b4")
            (nc.sync if tp % 2 == 0 else nc.scalar).dma_start_transpose(
                b4[:], b4t[:, 1024 * tp : 1024 * tp + 1024]
            )
            for th in range(2):
                t = 2 * tp + th
                po = psO.tile([128, 512], F32, tag="out")
                for jb in range(4):
                    B = 4 * t + jb
                    nc.tensor.matmul(
                        po[:, 128 * jb : 128 * jb + 128],
                        xt[:, 128 * B : 128 * B + 128],
                        b4[0:120, 4 * th + jb, :],
                        start=True, stop=True,
                    )
                stg = spool.tile([128, 512], F32, tag="ostage")
                stgs.append(stg)
                nc.vector.tensor_copy(stg[:], po[:])
                (nc.gpsimd if t % 2 == 0 else nc.sync).dma_start(
                    out3[:, 4 * t : 4 * t + 4, :],
                    stg[:].rearrange("c (r j) -> c r j", j=128),
                )

    nc.compile()
    return nc


_NC_CACHE = None


def _get_nc():
    global _NC_CACHE
    if _NC_CACHE is None:
        _NC_CACHE = build_kernel()
    return _NC_CACHE


def kernel(**inputs) -> np.ndarray:
    X = np.asarray(inputs["X"], np.float32)
    consts = _host_consts(
        {k: np.asarray(v, np.float32) for k, v in inputs.items() if k != "X"}
    )
    in_maps = []
    for core in range(NCORES):
        xs, mrow, emask, xt = _host_shard(X, core)
        m = dict(consts)
        m["x"] = xs
        m["mrow"] = mrow
        m["emask"] = emask
        m["xt"] = xt
        in_maps.append(m)

    nc = _get_nc()
    res = run_bass_kernel_spmd(nc, in_maps, core_ids=list(range(NCORES)))
    out = np.zeros((2, C, 128, 128), np.float32)
    for core in range(NCORES):
        b, ri = divmod(core, 4)
        # device rows are [t, jb]; cols are [dy, dx, yl, xl] (s-major)
        v = res.results[core]["out"].reshape(C, 8, 4, 2, 2, 2, 16)
        v = v.transpose(0, 1, 5, 3, 2, 6, 4).reshape(C, 32, 128)
        out[b, :, 32 * ri : 32 * ri + 32, :] = v
    return out


if __name__ == "__main__":
    print("smoke build only")
    build_kernel()
    print("build ok")


# revision 28
# speedup vs baseline: 1.0381x; 1.0381x over previous
"""CARAFE + MSGConv Trainium2 kernel (8 NeuronCores, spatial x batch sharding).

out[c, i, j] = sum_{p,q} W[5p+q, i, j] * Xpad[c, i//2 + p - 2, j//2 + q - 2]
 (CARAFE taps live at source resolution; identical for both subpixel parities).

Per core: one batch element (core//4) and a 16-source-row block (core%4).

v2 changes vs baseline:
 - dw tap loops split Vector (STT) || TensorEngine (diagonal-weight matmuls
   accumulating in PSUM), combined with one tensor_add then Silu.
 - b4 transpose moved from PE (transpose + PSUM->SBUF copy) to DMA xbar
   transpose (SBUF->SBUF, [128,128] bf16 blocks).
 - repl matmuls merged 2 row-pairs per call (16x N=200).
 - out matmuls write 4 jb blocks into one [128,512] PSUM bank; single
   staging copy per row-pair.
 - wcat softmax scaling on vector tensor_scalar (2x mode).
 - input DMA ordering: front-critical tensors first, xt/back-end consts later.
"""

import sys

sys.path.insert(0, "/opt/trn_rl_repo")

from contextlib import ExitStack

import ml_dtypes
import numpy as np

import concourse.bass as bass
import concourse.tile as tile
from concourse import bacc, library_config, mybir
from concourse.bass_utils import run_bass_kernel_spmd

BF16 = mybir.dt.bfloat16
F32 = mybir.dt.float32
I16 = mybir.dt.int16
AF = mybir.ActivationFunctionType
OP = mybir.AluOpType
nbf = ml_dtypes.bfloat16

C = 128
H = W = 64
NCORES = 8
XR = 24          # X shard rows (16 + 4 halo each side)
XW = 68          # padded width for dw slabs only
NEG = -30.0      # additive pre-activation mask; SiLU(-30) ~= -2.8e-12

# dw tap split: taps [0, NPE) on the TensorEngine, [NPE, 25) on Vector
NPE1 = 16        # dw1 PE taps
NPE2 = 16        # dw2 PE taps


# ======================================================================
# host-side parameter prep
# ======================================================================

def _fold_1x1(w, s):
    return (w[:, :, 0, 0] * s[:, None]).T.copy()


def _dw_taps(w, s, k):
    ch = w.shape[0]
    out = np.zeros((ch, 25), np.float32)
    off = (5 - k) // 2
    for ty in range(k):
        for tx in range(k):
            out[:, 5 * (ty + off) + (tx + off)] = w[:, 0, ty, tx] * s
    return out


def _host_consts(inputs):
    d = {}
    w_cv1 = _fold_1x1(inputs["comp_cv1_w"], inputs["comp_cv1_s"])
    b_cv1 = inputs["comp_cv1_b"].reshape(32, 1)
    w3 = _dw_taps(inputs["comp_dw3_w"], inputs["comp_dw3_s"], 3)
    w5 = _dw_taps(inputs["comp_dw5_w"], inputs["comp_dw5_s"], 5)
    w_dwp = np.tile(np.concatenate([w3, w5], 0), (4, 1))
    b_dwp = np.tile(
        np.concatenate([inputs["comp_dw3_b"], inputs["comp_dw5_b"]]), 4
    ).reshape(128, 1)
    w_px = _fold_1x1(inputs["comp_px_w"], inputs["comp_px_s"])
    b_px = inputs["comp_px_b"].reshape(64, 1)
    we = _fold_1x1(inputs["enc_cv1_w"], inputs["enc_cv1_s"])
    w_ecv1 = np.concatenate([we, np.ones((1, 50), np.float32)], 0)
    b_ecv1 = inputs["enc_cv1_b"].reshape(50, 1)
    e3 = _dw_taps(inputs["enc_dw3_w"], inputs["enc_dw3_s"], 3)
    e5 = _dw_taps(inputs["enc_dw5_w"], inputs["enc_dw5_s"], 5)
    w_edwp = np.tile(np.concatenate([e3, e5], 0), (2, 1))
    b_edwp = np.tile(
        np.concatenate([inputs["enc_dw3_b"], inputs["enc_dw5_b"]]), 2
    ).reshape(100, 1)
    wpx = _fold_1x1(inputs["enc_px_w"], inputs["enc_px_s"])
    w_epx = np.concatenate([wpx, inputs["enc_px_b"].reshape(1, 100)], 0)

    # packA bf16 [128, 510]: w_cv1 | w_px | w_ecv1 | w_epx_a | px2 masked
    # halves (even/odd dw1 groups) | epx_b masked halves (lo/hi e2p group)
    pa = np.zeros((128, 574), np.float32)
    pa[0:128, 0:32] = w_cv1
    pa[0:64, 32:96] = w_px
    pa[0:65, 96:146] = w_ecv1
    pa[0:50, 146:246] = w_epx[0:50]
    pa[50:51, 146:246] = w_epx[100:101]
    pa[0:32, 246:310] = w_px[32:64]      # g even (lhsT half [0:64] base 0/64)
    pa[64:96, 246:310] = w_px[32:64]
    pa[32:64, 310:374] = w_px[32:64]     # g odd
    pa[96:128, 310:374] = w_px[32:64]
    pa[0:50, 374:474] = w_epx[50:100]    # e2p group 0 (t < 4)
    pa[50:100, 474:574] = w_epx[50:100]  # e2p group 1 (t >= 4)
    d["packa"] = pa.astype(nbf)
    # packB f32 [128, 55]
    pb = np.zeros((128, 55), np.float32)
    pb[:, 0:25] = w_dwp
    pb[:, 25:26] = b_dwp
    pb[0:100, 26:51] = w_edwp
    pb[0:100, 51:52] = b_edwp
    pb[0:32, 52:53] = b_cv1
    pb[0:64, 53:54] = b_px
    pb[0:50, 54:55] = b_ecv1
    d["packb"] = pb

    # diagonal lhsT for the PE dw taps
    dg1 = np.zeros((128, NPE1 * 128), np.float32)
    for t in range(NPE1):
        dg1[np.arange(128), 128 * t + np.arange(128)] = w_dwp[:, t]
    d["diag1"] = dg1.astype(nbf)
    dg2 = np.zeros((100, NPE2 * 100), np.float32)
    for t in range(NPE2):
        dg2[np.arange(100), 100 * t + np.arange(100)] = w_edwp[:, t]
    d["diag2"] = dg2.astype(nbf)

    d["ones1"] = np.ones((1, 32), nbf)

    # replS [128, 4*128]: lhsT blocks per (jb, s); output pixel partition
    # within a jb block is s-major: m = 16*yl + xl, placed at psum
    # partitions 32s+m by the matmul's out slice.
    rp = np.zeros((128, 512), np.float32)
    for jb in range(4):
        for s_ in range(4):
            for yl in range(2):
                for xl in range(16):
                    rp[64 * yl + 16 * jb + xl,
                       128 * jb + 32 * s_ + 16 * yl + xl] = 1.0
    d["repl"] = rp.astype(nbf)

    # sidx [128, 200] int16, shared by all four t-pair scatter calls:
    # partition = s-major pixel-in-block (32s + 16yl + xl), data col
    # (th, jb, k) -> target slot 512*th + 128*jb + 20u + v (uv pitch 20,
    # cols 120..127 of each 128 block are pad for the xbar transpose).
    si = np.full((128, 200), -1, np.int16)
    for part in range(128):
        s_, m = divmod(part, 32)
        yl, xl = divmod(m, 16)
        for th in range(2):
            for jb in range(4):
                for k in range(25):
                    p, q = divmod(k, 5)
                    if not (0 <= 16 * jb + xl + q - 2 < 64):
                        continue
                    si[part, 100 * th + 25 * jb + k] = (
                        512 * th + 128 * jb + 20 * (yl + p) + (xl + q)
                    )
    d["sidx"] = si
    return d


def _host_shard(X, core):
    b, ri = divmod(core, 4)
    r0 = 16 * ri - 4
    xs = np.zeros((C, XR, W), np.float32)
    lo, hi = max(0, r0), min(H, r0 + XR)
    xs[:, lo - r0 : hi - r0, :] = X[b, :, lo:hi, :]
    mrow = np.zeros((1, XR, W), np.float32)
    for r in range(XR):
        if not (0 <= r0 + r < H):
            mrow[0, r, :] = NEG
    emask = np.zeros((1, 20, W), np.float32)
    for r in range(20):
        if not (0 <= (16 * ri - 2) + r < H):
            emask[0, r, :] = NEG
    xsb = xs.astype(nbf)
    # pre-transposed X slabs, one [120, 128] per block (column-padded)
    xsp = np.zeros((C, XR, XW), nbf)
    xsp[:, :, 2 : 2 + W] = xsb
    xt = np.zeros((120, 32 * 128), nbf)
    for B in range(32):
        t, jb = divmod(B, 4)
        slab = xsp[:, 2 * t + 2 : 2 * t + 8, 16 * jb : 16 * jb + 20]
        xt[:, 128 * B : 128 * B + 128] = slab.reshape(C, 120).T
    return (
        xsb.reshape(C, XR * W),
        mrow.reshape(1, XR * W).astype(nbf),
        emask.reshape(1, 20 * W).astype(nbf),
        xt,
    )


# ======================================================================
# device kernel
# ======================================================================

def build_kernel():
    nc = bacc.Bacc(
        "TRN2",
        target_bir_lowering=False,
        debug=False,
        enable_asserts=False,
        num_devices=NCORES,
    )

    def din(name, shape, dt):
        return nc.dram_tensor(name, list(shape), dt, kind="ExternalInput").ap()

    x_d = din("x", (128, XR * W), BF16)
    xt_d = din("xt", (120, 32 * 128), BF16)
    mrow_d = din("mrow", (1, XR * W), BF16)
    emask_d = din("emask", (1, 20 * W), BF16)
    ones1_d = din("ones1", (1, 32), BF16)
    packa_d = din("packa", (128, 574), BF16)
    packb_d = din("packb", (128, 55), F32)
    diag1_d = din("diag1", (128, NPE1 * 128), BF16)
    diag2_d = din("diag2", (100, NPE2 * 100), BF16)
    repl_d = din("repl", (128, 512), BF16)
    sidx_d = din("sidx", (128, 200), I16)
    out_d = nc.dram_tensor("out", [128, 32 * 128], F32, kind="ExternalOutput").ap()
    out3 = out_d.rearrange("c (r j) -> c r j", j=128)

    with tile.TileContext(nc) as tc, ExitStack() as ctx:
        cpool = ctx.enter_context(tc.tile_pool(name="consts", bufs=1))
        work = ctx.enter_context(tc.tile_pool(name="work", bufs=1))
        psB = ctx.enter_context(tc.tile_pool(name="psB", bufs=2, space="PSUM"))
        spool = ctx.enter_context(tc.tile_pool(name="stage", bufs=3))
        bpool = ctx.enter_context(tc.tile_pool(name="b4s", bufs=6))
        psA_cm = tc.tile_pool(name="psA", bufs=2, space="PSUM")
        psA = psA_cm.__enter__()
        psD_cm = tc.tile_pool(name="psD", bufs=1, space="PSUM")
        psD = psD_cm.__enter__()

        nc.gpsimd.load_library(library_config.local_scatter)

        def cload(ap_d, shape, dt, eng=None):
            t = cpool.tile(list(shape), dt, tag=ap_d.tensor.name)
            (eng or nc.sync).dma_start(t[:], ap_d)
            return t

        # front-critical loads first, in queue-program order per engine
        xb = cpool.tile([128, XR * W], BF16, tag="x")
        for ch, eng in enumerate((nc.sync, nc.scalar, nc.gpsimd)):
            eng.dma_start(
                xb[:, 8 * W * ch : 8 * W * (ch + 1)],
                x_d[:, 8 * W * ch : 8 * W * (ch + 1)],
            )
        packa = cload(packa_d, (128, 574), BF16)
        mrow = cload(mrow_d, (1, XR * W), BF16, eng=nc.scalar)
        packb = cload(packb_d, (128, 55), F32, eng=nc.scalar)
        ones1 = cload(ones1_d, (1, 32), BF16, eng=nc.scalar)
        diag1 = cload(diag1_d, (128, NPE1 * 128), BF16)
        diag2 = cload(diag2_d, (100, NPE2 * 100), BF16)
        # back-end consts (needed later) on the gpsimd software queue
        repl = cload(repl_d, (128, 512), BF16, eng=nc.gpsimd)
        sidx = cload(sidx_d, (128, 200), I16, eng=nc.gpsimd)
        xt = cpool.tile([120, 32 * 128], BF16, tag="xt")

        w_cv1 = packa[0:128, 0:32]
        w_px = packa[0:64, 32:96]
        w_ecv1 = packa[0:65, 96:146]
        w_epx = packa[0:101, 146:246]
        w_dwp = packb[0:128, 0:25]
        b_dwp = packb[0:128, 25:26]
        w_edwp = packb[0:100, 26:51]
        b_edwp = packb[0:100, 51:52]
        b_cv1 = packb[0:32, 52:53]
        b_px = packb[0:64, 53:54]
        b_ecv1 = packb[0:50, 54:55]

        # warmup: trigger the local_scatter ucode library load early so it
        # overlaps the conv front instead of stalling the first real scatter
        warm = work.tile([16, 16], BF16)
        nc.gpsimd.local_scatter(
            warm[:], packa[0:16, 0:2], sidx[:][0:16, 0:2],
            channels=16, num_elems=16, num_idxs=2,
        )

        # persistent working tensors (all 64-wide / contiguous)
        x12 = work.tile([64, XR * W], BF16)        # x1 (0:32) + x2 (32:64)
        enc_in = work.tile([65, 20 * W], BF16)     # px out + mask row
        e1c = work.tile([51, 20 * W], BF16)        # enc cv1 out + ones row
        x1p = work.tile([128, 9 * XW + 8], BF16)   # packed x1 (68-pitch)
        e1p = work.tile([100, 12 * XW + 8], BF16)  # packed enc x1 (68-pitch)
        ET = work.tile([128, 800], BF16)
        expv = work.tile([128, 800], BF16)         # [s][t][k]
        S = work.tile([128, 32], F32)
        R = work.tile([128, 32], F32)
        wcat = work.tile([128, 800], BF16)         # [t][s][k]
        dall = work.tile([128, 3200], BF16)
        b4t = work.tile([128, 8 * 512], BF16)      # per t: [4jb x 128]

        xb3 = xb[:].rearrange("p (r c) -> p r c", c=W)
        x12_3 = x12[:].rearrange("p (r c) -> p r c", c=W)
        e1c3 = e1c[:].rearrange("p (r c) -> p r c", c=W)
        x1p3 = x1p[:, 0 : 9 * XW].rearrange("p (r c) -> p r c", c=XW)
        e1p3 = e1p[:, 0 : 12 * XW].rearrange("p (r c) -> p r c", c=XW)
        ET3 = ET[:].rearrange("p (t e) -> p t e", e=100)
        exp3 = expv[:].rearrange("p (s t k) -> p s t k", s=4, t=8)

        # zero only the dw-slab pad columns (cols 0:2 and 66:68)
        nc.vector.memset(x1p[:, 9 * XW : 9 * XW + 8], 0.0)
        nc.vector.memset(e1p[:, 12 * XW : 12 * XW + 8], 0.0)
        nc.vector.memset(x1p3[:, :, 0:2], 0.0)
        nc.vector.memset(x1p3[:, :, 66:68], 0.0)
        nc.vector.memset(e1p3[:, :, 0:2], 0.0)
        nc.vector.memset(e1p3[:, :, 66:68], 0.0)
        nc.vector.memset(e1c[:], 1.0)
        nc.sync.dma_start(enc_in[64:65, :], emask_d)

        # ---- comp cv1: 1x1 conv 128->32 (+ SiLU + out-of-image row mask)
        for ch in range(3):
            ps = psA.tile([32, 512], F32, tag="convps")
            nc.tensor.matmul(
                ps[:], w_cv1, xb[:, 512 * ch : 512 * (ch + 1)],
                start=True, stop=False,
            )
            nc.tensor.matmul(
                ps[:], ones1[:], mrow[:, 512 * ch : 512 * (ch + 1)],
                start=False, stop=True,
            )
            nc.scalar.activation(
                x12[0:32, 512 * ch : 512 * (ch + 1)], ps[:],
                AF.Silu, bias=b_cv1,
            )

        # ---- comp dw3/dw5 (unified 5x5 taps, rows packed 4x32)
        # taps [0, NPE1) on the TensorEngine (diag lhsT, PSUM accumulate),
        # taps [NPE1, 25) on Vector (STT chain); combined + Silu.
        for g, eng in enumerate((nc.sync, nc.scalar, nc.gpsimd, nc.sync)):
            eng.dma_start(
                x1p3[32 * g : 32 * g + 32, 0:9, 2 : 2 + W],
                x12_3[0:32, 5 * g : 5 * g + 9, :],
            )
        FS = 5 * XW                    # 340
        ps1 = psD.tile([128, FS], F32, tag="dw1")
        for t in range(NPE1):
            ty, tx = divmod(t, 5)
            nc.tensor.matmul(
                ps1[:], diag1[:, 128 * t : 128 * t + 128],
                x1p[:, ty * XW + tx : ty * XW + tx + FS],
                start=(t == 0), stop=(t == NPE1 - 1),
            )
        acc_a = work.tile([128, FS], BF16)
        av = acc_a[:]
        for i, t in enumerate(range(NPE1, 25)):
            ty, tx = divmod(t, 5)
            sv = x1p[:, ty * XW + tx : ty * XW + tx + FS]
            if i == 0:
                nc.vector.tensor_scalar(av, sv, w_dwp[:, t : t + 1], None, OP.mult)
            else:
                nc.vector.scalar_tensor_tensor(
                    av, sv, w_dwp[:, t : t + 1], av, OP.mult, OP.add
                )
        nc.vector.tensor_add(av, av, ps1[:])
        x2q = work.tile([128, 5 * W], BF16)
        nc.scalar.activation(
            x2q[:].rearrange("p (r c) -> p r c", c=W),
            acc_a[:].rearrange("p (r c) -> p r c", c=XW)[:, 0:5, 0:W],
            AF.Silu, bias=b_dwp,
        )

        # ---- comp px: 1x1 conv 64->64 (+ SiLU), split K: x1 from x12 rows,
        # x2 read directly from the packed x2p slabs (no writeback DMA)
        w_px1 = packa[0:32, 32:96]
        for g in range(4):
            ps = psA.tile([64, 5 * W], F32, tag="convps")
            nc.tensor.matmul(
                ps[:], w_px1,
                x12[0:32, (2 + 5 * g) * W : (7 + 5 * g) * W],
                start=True, stop=False,
            )
            base, cols = 64 * (g // 2), (246 if g % 2 == 0 else 310)
            nc.tensor.matmul(
                ps[:], packa[base : base + 64, cols : cols + 64],
                x2q[base : base + 64, :],
                start=False, stop=True,
            )
            nc.scalar.activation(
                enc_in[0:64, 5 * g * W : (5 * g + 5) * W], ps[:],
                AF.Silu, bias=b_px,
            )

        # ---- enc cv1: 1x1 conv 64->50 (+ SiLU, mask row rides K=65);
        # e1p pack DMAs issued as soon as their source rows are done
        for r0, nr in ((0, 8), (8, 8), (16, 4)):
            ps = psA.tile([50, 512], F32, tag="convps")
            nc.tensor.matmul(
                ps[:, : nr * W], w_ecv1,
                enc_in[0:65, r0 * W : (r0 + nr) * W],
                start=True, stop=True,
            )
            nc.scalar.activation(
                e1c[0:50, r0 * W : (r0 + nr) * W], ps[:, : nr * W],
                AF.Silu, bias=b_ecv1,
            )
            if r0 == 8:
                nc.sync.dma_start(
                    e1p3[0:50, 0:12, 2 : 2 + W], e1c3[0:50, 0:12, :]
                )
        nc.scalar.dma_start(
            e1p3[50:100, 0:12, 2 : 2 + W], e1c3[0:50, 8:20, :]
        )
        FS2 = 8 * XW                   # 544
        HF = FS2 // 2                  # 272
        ps2a = psD.tile([100, HF], F32, tag="dw2a")
        ps2b = psD.tile([100, HF], F32, tag="dw2b")
        for t in range(NPE2):
            ty, tx = divmod(t, 5)
            base = ty * XW + tx
            nc.tensor.matmul(
                ps2a[:], diag2[:, 100 * t : 100 * t + 100],
                e1p[0:100, base : base + HF],
                start=(t == 0), stop=(t == NPE2 - 1),
            )
        for t in range(NPE2):
            ty, tx = divmod(t, 5)
            base = ty * XW + tx
            nc.tensor.matmul(
                ps2b[:], diag2[:, 100 * t : 100 * t + 100],
                e1p[0:100, base + HF : base + FS2],
                start=(t == 0), stop=(t == NPE2 - 1),
            )
        acc2_a = work.tile([100, FS2], BF16)
        av2 = acc2_a[:]
        for i, t in enumerate(range(NPE2, 25)):
            ty, tx = divmod(t, 5)
            sv = e1p[:, ty * XW + tx : ty * XW + tx + FS2]
            if i == 0:
                nc.vector.tensor_scalar(av2, sv, w_edwp[:, t : t + 1], None, OP.mult)
            else:
                nc.vector.scalar_tensor_tensor(
                    av2, sv, w_edwp[:, t : t + 1], av2, OP.mult, OP.add
                )
        nc.vector.tensor_add(acc2_a[:, 0:HF], acc2_a[:, 0:HF], ps2a[:])
        nc.vector.tensor_add(acc2_a[:, HF:FS2], acc2_a[:, HF:FS2], ps2b[:])
        e2q = work.tile([100, 8 * W], BF16)
        nc.scalar.activation(
            e2q[:].rearrange("p (r c) -> p r c", c=W),
            acc2_a[:].rearrange("p (r c) -> p r c", c=XW)[:, 0:8, 0:W],
            AF.Silu, bias=b_edwp,
        )

        # xt load (needed by the out matmuls from ~mid-kernel only;
        # emitting it here avoids false semaphore deps in the conv front)
        for ch, eng in enumerate((nc.sync, nc.scalar)):
            eng.dma_start(
                xt[:, 2048 * ch : 2048 * (ch + 1)],
                xt_d[:, 2048 * ch : 2048 * (ch + 1)],
            )

        # ---- enc px (transposed output: M = 128 pixels per row-pair),
        # split K: e1 + bias row from e1c, e2 direct from e2p slabs
        w_epx_a = packa[0:51, 146:246]
        for t in range(8):
            g, lr = divmod(2 * t, 8)
            ps = psA.tile([128, 100], F32, tag="convps")
            nc.tensor.matmul(
                ps[:], e1c[0:51, (2 + 2 * t) * W : (4 + 2 * t) * W],
                w_epx_a, start=True, stop=False,
            )
            cols = 374 if g == 0 else 474
            nc.tensor.matmul(
                ps[:], e2q[0:100, lr * W : (lr + 2) * W],
                packa[0:100, cols : cols + 100],
                start=False, stop=True,
            )
            nc.scalar.activation(ET[:, 100 * t : 100 * t + 100], ps[:], AF.Silu)

        # ---- softmax over 25 taps per subposition (no max-subtraction)
        for s in range(4):
            nc.scalar.activation(exp3[:, s], ET3[:, :, s::4], AF.Exp)
            nc.vector.tensor_reduce(
                S[:, 8 * s : 8 * s + 8], exp3[:, s], mybir.AxisListType.X, OP.add
            )
        nc.vector.reciprocal(R[:], S[:])
        psD_cm.__exit__(None, None, None)
        psA_cm.__exit__(None, None, None)
        psO = ctx.enter_context(tc.tile_pool(name="psO", bufs=3, space="PSUM"))

        # normalized weights, s-major: wcat[p, 200s + 25t + k]
        # = exp3[p, s, t, k] * R[p, 8s+t]
        R3 = R[:].rearrange("p (s u) -> p s u", s=4)
        wcat4 = wcat[:].rearrange("p (s t k) -> p s t k", s=4, t=8)
        for t in range(8):
            nc.vector.tensor_tensor(
                wcat4[:, :, t],
                exp3[:, :, t],
                R3[:, :, t : t + 1].to_broadcast((128, 4, 25)),
                OP.mult,
            )

        # repl matmuls: per (jb, s) one [32, 200] output at psum partition
        # offset 32s (s-major pixel packing); then cast into dall2 with the
        # (tp, th, jb, k) column interleave the t-pair scatters consume.
        dall2v = dall[:, 0:800].rearrange(
            "p (tp th j k) -> p tp th j k", tp=4, th=2, j=4
        )
        for jb in range(4):
            ps = psB.tile([128, 200], F32, tag="repl")
            for s_ in range(4):
                nc.tensor.matmul(
                    ps[32 * s_ : 32 * s_ + 32, :],
                    repl[:, 128 * jb + 32 * s_ : 128 * jb + 32 * s_ + 32],
                    wcat[:, 200 * s_ : 200 * s_ + 200],
                    start=True, stop=True,
                    tile_position=(0, 32 * s_),
                )
            src3 = ps[:].rearrange("p (tp th k) -> p tp th k", tp=4, th=2)
            if jb % 2 == 0:
                nc.vector.tensor_copy(dall2v[:, :, :, jb], src3)
            else:
                nc.scalar.copy(dall2v[:, :, :, jb], src3)

        # scatters first (4 s-compacted calls, one t-pair each: 200 idx,
        # 1024 out) so the gpsimd queue never stalls behind downstream
        # DMAs; then per t-pair: one chunked DMA transpose -> per row-pair
        # 4 matmuls into a [128,512] PSUM bank -> 1 straight staging copy
        # (s-major pixel columns; the host unpermutes) -> out DMA.
        for tp in range(4):
            nc.gpsimd.local_scatter(
                b4t[:, 1024 * tp : 1024 * tp + 1024],
                dall[:, 200 * tp : 200 * tp + 200],
                sidx[:],
                channels=128, num_elems=1024, num_idxs=200,
            )
        stgs = []
        for tp in range(4):
            b4 = bpool.tile([128, 8, 128], BF16, tag="b4")
            (nc.sync if tp % 2 == 0 else nc.scalar).dma_start_transpose(
                b4[:], b4t[:, 1024 * tp : 1024 * tp + 1024]
            )
            for th in range(2):
                t = 2 * tp + th
                po = psO.tile([128, 512], F32, tag="out")
                for jb in range(4):
                    B = 4 * t + jb
                    nc.tensor.matmul(
                        po[:, 128 * jb : 128 * jb + 128],
                        xt[:, 128 * B : 128 * B + 128],
                        b4[0:120, 4 * th + jb, :],
                        start=True, stop=True,
                    )
                stg = spool.tile([128, 512], F32, tag="ostage")
                stgs.append(stg)
                if t % 2 == 0:
                    nc.vector.tensor_copy(stg[:], po[:])
                else:
                    nc.scalar.copy(stg[:], po[:])
                (nc.scalar if t % 2 == 0 else nc.sync).dma_start(
                    out3[:, 4 * t : 4 * t + 4, :],
                    stg[:].rearrange("c (r j) -> c r j", j=128),
                )

    nc.compile()
    return nc


_NC_CACHE = None


def _get_nc():
    global _NC_CACHE
    if _NC_CACHE is None:
        _NC_CACHE = build_kernel()
    return _NC_CACHE


def kernel(**inputs) -> np.ndarray:
    X = np.asarray(inputs["X"], np.float32)
    consts = _host_consts(
        {k: np.asarray(v, np.float32) for k, v in inputs.items() if k != "X"}
    )
    in_maps = []
    for core in range(NCORES):
        xs, mrow, emask, xt = _host_shard(X, core)
        m = dict(consts)
        m["x"] = xs
        m["mrow"] = mrow
        m["emask"] = emask
        m["xt"] = xt
        in_maps.append(m)

    nc = _get_nc()
    res = run_bass_kernel_spmd(nc, in_maps, core_ids=list(range(NCORES)))
    out = np.zeros((2, C, 128, 128), np.float32)
    for core in range(NCORES):
        b, ri = divmod(core, 4)
        # device rows are [t, jb]; cols are [dy, dx, yl, xl] (s-major)
        v = res.results[core]["out"].reshape(C, 8, 4, 2, 2, 2, 16)
        v = v.transpose(0, 1, 5, 3, 2, 6, 4).reshape(C, 32, 128)
        out[b, :, 32 * ri : 32 * ri + 32, :] = v
    return out


if __name__ == "__main__":
    print("smoke build only")
    build_kernel()
    print("build ok")


# revision 29
# speedup vs baseline: 1.0458x; 1.0074x over previous
"""CARAFE + MSGConv Trainium2 kernel (8 NeuronCores, spatial x batch sharding).

out[c, i, j] = sum_{p,q} W[5p+q, i, j] * Xpad[c, i//2 + p - 2, j//2 + q - 2]
 (CARAFE taps live at source resolution; identical for both subpixel parities).

Per core: one batch element (core//4) and a 16-source-row block (core%4).

v2 changes vs baseline:
 - dw tap loops split Vector (STT) || TensorEngine (diagonal-weight matmuls
   accumulating in PSUM), combined with one tensor_add then Silu.
 - b4 transpose moved from PE (transpose + PSUM->SBUF copy) to DMA xbar
   transpose (SBUF->SBUF, [128,128] bf16 blocks).
 - repl matmuls merged 2 row-pairs per call (16x N=200).
 - out matmuls write 4 jb blocks into one [128,512] PSUM bank; single
   staging copy per row-pair.
 - wcat softmax scaling on vector tensor_scalar (2x mode).
 - input DMA ordering: front-critical tensors first, xt/back-end consts later.
"""

import sys

sys.path.insert(0, "/opt/trn_rl_repo")

from contextlib import ExitStack

import ml_dtypes
import numpy as np

import concourse.bass as bass
import concourse.tile as tile
from concourse import bacc, library_config, mybir
from concourse.bass_utils import run_bass_kernel_spmd

BF16 = mybir.dt.bfloat16
F32 = mybir.dt.float32
I16 = mybir.dt.int16
AF = mybir.ActivationFunctionType
OP = mybir.AluOpType
nbf = ml_dtypes.bfloat16

C = 128
H = W = 64
NCORES = 8
XR = 24          # X shard rows (16 + 4 halo each side)
XW = 68          # padded width for dw slabs only
NEG = -30.0      # additive pre-activation mask; SiLU(-30) ~= -2.8e-12

# dw tap split: taps [0, NPE) on the TensorEngine, [NPE, 25) on Vector
NPE1 = 17        # dw1 PE taps
NPE2 = 16        # dw2 PE taps


# ======================================================================
# host-side parameter prep
# ======================================================================

def _fold_1x1(w, s):
    return (w[:, :, 0, 0] * s[:, None]).T.copy()


def _dw_taps(w, s, k):
    ch = w.shape[0]
    out = np.zeros((ch, 25), np.float32)
    off = (5 - k) // 2
    for ty in range(k):
        for tx in range(k):
            out[:, 5 * (ty + off) + (tx + off)] = w[:, 0, ty, tx] * s
    return out


def _host_consts(inputs):
    d = {}
    w_cv1 = _fold_1x1(inputs["comp_cv1_w"], inputs["comp_cv1_s"])
    b_cv1 = inputs["comp_cv1_b"].reshape(32, 1)
    w3 = _dw_taps(inputs["comp_dw3_w"], inputs["comp_dw3_s"], 3)
    w5 = _dw_taps(inputs["comp_dw5_w"], inputs["comp_dw5_s"], 5)
    w_dwp = np.tile(np.concatenate([w3, w5], 0), (4, 1))
    b_dwp = np.tile(
        np.concatenate([inputs["comp_dw3_b"], inputs["comp_dw5_b"]]), 4
    ).reshape(128, 1)
    w_px = _fold_1x1(inputs["comp_px_w"], inputs["comp_px_s"])
    b_px = inputs["comp_px_b"].reshape(64, 1)
    we = _fold_1x1(inputs["enc_cv1_w"], inputs["enc_cv1_s"])
    w_ecv1 = np.concatenate([we, np.ones((1, 50), np.float32)], 0)
    b_ecv1 = inputs["enc_cv1_b"].reshape(50, 1)
    e3 = _dw_taps(inputs["enc_dw3_w"], inputs["enc_dw3_s"], 3)
    e5 = _dw_taps(inputs["enc_dw5_w"], inputs["enc_dw5_s"], 5)
    w_edwp = np.tile(np.concatenate([e3, e5], 0), (2, 1))
    b_edwp = np.tile(
        np.concatenate([inputs["enc_dw3_b"], inputs["enc_dw5_b"]]), 2
    ).reshape(100, 1)
    wpx = _fold_1x1(inputs["enc_px_w"], inputs["enc_px_s"])
    w_epx = np.concatenate([wpx, inputs["enc_px_b"].reshape(1, 100)], 0)

    # packA bf16 [128, 510]: w_cv1 | w_px | w_ecv1 | w_epx_a | px2 masked
    # halves (even/odd dw1 groups) | epx_b masked halves (lo/hi e2p group)
    pa = np.zeros((128, 574), np.float32)
    pa[0:128, 0:32] = w_cv1
    pa[0:64, 32:96] = w_px
    pa[0:65, 96:146] = w_ecv1
    pa[0:50, 146:246] = w_epx[0:50]
    pa[50:51, 146:246] = w_epx[100:101]
    pa[0:32, 246:310] = w_px[32:64]      # g even (lhsT half [0:64] base 0/64)
    pa[64:96, 246:310] = w_px[32:64]
    pa[32:64, 310:374] = w_px[32:64]     # g odd
    pa[96:128, 310:374] = w_px[32:64]
    pa[0:50, 374:474] = w_epx[50:100]    # e2p group 0 (t < 4)
    pa[50:100, 474:574] = w_epx[50:100]  # e2p group 1 (t >= 4)
    d["packa"] = pa.astype(nbf)
    # packB f32 [128, 55]
    pb = np.zeros((128, 55), np.float32)
    pb[:, 0:25] = w_dwp
    pb[:, 25:26] = b_dwp
    pb[0:100, 26:51] = w_edwp
    pb[0:100, 51:52] = b_edwp
    pb[0:32, 52:53] = b_cv1
    pb[0:64, 53:54] = b_px
    pb[0:50, 54:55] = b_ecv1
    d["packb"] = pb

    # diagonal lhsT for the PE dw taps
    dg1 = np.zeros((128, NPE1 * 128), np.float32)
    for t in range(NPE1):
        dg1[np.arange(128), 128 * t + np.arange(128)] = w_dwp[:, t]
    d["diag1"] = dg1.astype(nbf)
    dg2 = np.zeros((100, NPE2 * 100), np.float32)
    for t in range(NPE2):
        dg2[np.arange(100), 100 * t + np.arange(100)] = w_edwp[:, t]
    d["diag2"] = dg2.astype(nbf)

    d["ones1"] = np.ones((1, 32), nbf)

    # replS [128, 4*128]: lhsT blocks per (jb, s); output pixel partition
    # within a jb block is s-major: m = 16*yl + xl, placed at psum
    # partitions 32s+m by the matmul's out slice.
    rp = np.zeros((128, 512), np.float32)
    for jb in range(4):
        for s_ in range(4):
            for yl in range(2):
                for xl in range(16):
                    rp[64 * yl + 16 * jb + xl,
                       128 * jb + 32 * s_ + 16 * yl + xl] = 1.0
    d["repl"] = rp.astype(nbf)

    # sidx [128, 200] int16, shared by all four t-pair scatter calls:
    # partition = s-major pixel-in-block (32s + 16yl + xl), data col
    # (th, jb, k) -> target slot 512*th + 128*jb + 20u + v (uv pitch 20,
    # cols 120..127 of each 128 block are pad for the xbar transpose).
    si = np.full((128, 200), -1, np.int16)
    for part in range(128):
        s_, m = divmod(part, 32)
        yl, xl = divmod(m, 16)
        for th in range(2):
            for jb in range(4):
                for k in range(25):
                    p, q = divmod(k, 5)
                    if not (0 <= 16 * jb + xl + q - 2 < 64):
                        continue
                    si[part, 100 * th + 25 * jb + k] = (
                        512 * th + 128 * jb + 20 * (yl + p) + (xl + q)
                    )
    d["sidx"] = si
    return d


def _host_shard(X, core):
    b, ri = divmod(core, 4)
    r0 = 16 * ri - 4
    xs = np.zeros((C, XR, W), np.float32)
    lo, hi = max(0, r0), min(H, r0 + XR)
    xs[:, lo - r0 : hi - r0, :] = X[b, :, lo:hi, :]
    mrow = np.zeros((1, XR, W), np.float32)
    for r in range(XR):
        if not (0 <= r0 + r < H):
            mrow[0, r, :] = NEG
    emask = np.zeros((1, 20, W), np.float32)
    for r in range(20):
        if not (0 <= (16 * ri - 2) + r < H):
            emask[0, r, :] = NEG
    xsb = xs.astype(nbf)
    # pre-transposed X slabs, one [120, 128] per block (column-padded)
    xsp = np.zeros((C, XR, XW), nbf)
    xsp[:, :, 2 : 2 + W] = xsb
    xt = np.zeros((120, 32 * 128), nbf)
    for B in range(32):
        t, jb = divmod(B, 4)
        slab = xsp[:, 2 * t + 2 : 2 * t + 8, 16 * jb : 16 * jb + 20]
        xt[:, 128 * B : 128 * B + 128] = slab.reshape(C, 120).T
    return (
        xsb.reshape(C, XR * W),
        mrow.reshape(1, XR * W).astype(nbf),
        emask.reshape(1, 20 * W).astype(nbf),
        xt,
    )


# ======================================================================
# device kernel
# ======================================================================

def build_kernel():
    nc = bacc.Bacc(
        "TRN2",
        target_bir_lowering=False,
        debug=False,
        enable_asserts=False,
        num_devices=NCORES,
    )

    def din(name, shape, dt):
        return nc.dram_tensor(name, list(shape), dt, kind="ExternalInput").ap()

    x_d = din("x", (128, XR * W), BF16)
    xt_d = din("xt", (120, 32 * 128), BF16)
    mrow_d = din("mrow", (1, XR * W), BF16)
    emask_d = din("emask", (1, 20 * W), BF16)
    ones1_d = din("ones1", (1, 32), BF16)
    packa_d = din("packa", (128, 574), BF16)
    packb_d = din("packb", (128, 55), F32)
    diag1_d = din("diag1", (128, NPE1 * 128), BF16)
    diag2_d = din("diag2", (100, NPE2 * 100), BF16)
    repl_d = din("repl", (128, 512), BF16)
    sidx_d = din("sidx", (128, 200), I16)
    out_d = nc.dram_tensor("out", [128, 32 * 128], F32, kind="ExternalOutput").ap()
    out3 = out_d.rearrange("c (r j) -> c r j", j=128)

    with tile.TileContext(nc) as tc, ExitStack() as ctx:
        cpool = ctx.enter_context(tc.tile_pool(name="consts", bufs=1))
        work = ctx.enter_context(tc.tile_pool(name="work", bufs=1))
        psB = ctx.enter_context(tc.tile_pool(name="psB", bufs=2, space="PSUM"))
        spool = ctx.enter_context(tc.tile_pool(name="stage", bufs=4))
        bpool = ctx.enter_context(tc.tile_pool(name="b4s", bufs=6))
        psA_cm = tc.tile_pool(name="psA", bufs=2, space="PSUM")
        psA = psA_cm.__enter__()
        psD_cm = tc.tile_pool(name="psD", bufs=1, space="PSUM")
        psD = psD_cm.__enter__()

        nc.gpsimd.load_library(library_config.local_scatter)

        def cload(ap_d, shape, dt, eng=None):
            t = cpool.tile(list(shape), dt, tag=ap_d.tensor.name)
            (eng or nc.sync).dma_start(t[:], ap_d)
            return t

        # front-critical loads first, in queue-program order per engine
        xb = cpool.tile([128, XR * W], BF16, tag="x")
        for ch, eng in enumerate((nc.sync, nc.scalar, nc.gpsimd)):
            eng.dma_start(
                xb[:, 8 * W * ch : 8 * W * (ch + 1)],
                x_d[:, 8 * W * ch : 8 * W * (ch + 1)],
            )
        packa = cload(packa_d, (128, 574), BF16)
        mrow = cload(mrow_d, (1, XR * W), BF16, eng=nc.scalar)
        packb = cload(packb_d, (128, 55), F32, eng=nc.scalar)
        ones1 = cload(ones1_d, (1, 32), BF16, eng=nc.scalar)
        diag1 = cload(diag1_d, (128, NPE1 * 128), BF16)
        diag2 = cload(diag2_d, (100, NPE2 * 100), BF16)
        # back-end consts (needed later) on the gpsimd software queue
        repl = cload(repl_d, (128, 512), BF16, eng=nc.gpsimd)
        sidx = cload(sidx_d, (128, 200), I16, eng=nc.gpsimd)
        xt = cpool.tile([120, 32 * 128], BF16, tag="xt")

        w_cv1 = packa[0:128, 0:32]
        w_px = packa[0:64, 32:96]
        w_ecv1 = packa[0:65, 96:146]
        w_epx = packa[0:101, 146:246]
        w_dwp = packb[0:128, 0:25]
        b_dwp = packb[0:128, 25:26]
        w_edwp = packb[0:100, 26:51]
        b_edwp = packb[0:100, 51:52]
        b_cv1 = packb[0:32, 52:53]
        b_px = packb[0:64, 53:54]
        b_ecv1 = packb[0:50, 54:55]

        # warmup: trigger the local_scatter ucode library load early so it
        # overlaps the conv front instead of stalling the first real scatter
        warm = work.tile([16, 16], BF16)
        nc.gpsimd.local_scatter(
            warm[:], packa[0:16, 0:2], sidx[:][0:16, 0:2],
            channels=16, num_elems=16, num_idxs=2,
        )

        # persistent working tensors (all 64-wide / contiguous)
        x12 = work.tile([64, XR * W], BF16)        # x1 (0:32) + x2 (32:64)
        enc_in = work.tile([65, 20 * W], BF16)     # px out + mask row
        e1c = work.tile([51, 20 * W], BF16)        # enc cv1 out + ones row
        x1p = work.tile([128, 9 * XW + 8], BF16)   # packed x1 (68-pitch)
        e1p = work.tile([100, 12 * XW + 8], BF16)  # packed enc x1 (68-pitch)
        ET = work.tile([128, 800], BF16)
        expv = work.tile([128, 800], BF16)         # [s][t][k]
        S = work.tile([128, 32], F32)
        R = work.tile([128, 32], F32)
        wcat = work.tile([128, 800], BF16)         # [t][s][k]
        dall = work.tile([128, 3200], BF16)
        b4t = work.tile([128, 8 * 512], BF16)      # per t: [4jb x 128]

        xb3 = xb[:].rearrange("p (r c) -> p r c", c=W)
        x12_3 = x12[:].rearrange("p (r c) -> p r c", c=W)
        e1c3 = e1c[:].rearrange("p (r c) -> p r c", c=W)
        x1p3 = x1p[:, 0 : 9 * XW].rearrange("p (r c) -> p r c", c=XW)
        e1p3 = e1p[:, 0 : 12 * XW].rearrange("p (r c) -> p r c", c=XW)
        ET3 = ET[:].rearrange("p (t e) -> p t e", e=100)
        exp3 = expv[:].rearrange("p (s t k) -> p s t k", s=4, t=8)

        # zero only the dw-slab pad columns (cols 0:2 and 66:68)
        nc.vector.memset(x1p[:, 9 * XW : 9 * XW + 8], 0.0)
        nc.vector.memset(e1p[:, 12 * XW : 12 * XW + 8], 0.0)
        nc.vector.memset(x1p3[:, :, 0:2], 0.0)
        nc.vector.memset(x1p3[:, :, 66:68], 0.0)
        nc.vector.memset(e1p3[:, :, 0:2], 0.0)
        nc.vector.memset(e1p3[:, :, 66:68], 0.0)
        nc.vector.memset(e1c[:], 1.0)
        nc.sync.dma_start(enc_in[64:65, :], emask_d)

        # ---- comp cv1: 1x1 conv 128->32 (+ SiLU + out-of-image row mask)
        for ch in range(3):
            ps = psA.tile([32, 512], F32, tag="convps")
            nc.tensor.matmul(
                ps[:], w_cv1, xb[:, 512 * ch : 512 * (ch + 1)],
                start=True, stop=False,
            )
            nc.tensor.matmul(
                ps[:], ones1[:], mrow[:, 512 * ch : 512 * (ch + 1)],
                start=False, stop=True,
            )
            nc.scalar.activation(
                x12[0:32, 512 * ch : 512 * (ch + 1)], ps[:],
                AF.Silu, bias=b_cv1,
            )

        # ---- comp dw3/dw5 (unified 5x5 taps, rows packed 4x32)
        # taps [0, NPE1) on the TensorEngine (diag lhsT, PSUM accumulate),
        # taps [NPE1, 25) on Vector (STT chain); combined + Silu.
        for g, eng in enumerate((nc.sync, nc.scalar, nc.gpsimd, nc.sync)):
            eng.dma_start(
                x1p3[32 * g : 32 * g + 32, 0:9, 2 : 2 + W],
                x12_3[0:32, 5 * g : 5 * g + 9, :],
            )
        FS = 5 * XW                    # 340
        ps1 = psD.tile([128, FS], F32, tag="dw1")
        for t in range(NPE1):
            ty, tx = divmod(t, 5)
            nc.tensor.matmul(
                ps1[:], diag1[:, 128 * t : 128 * t + 128],
                x1p[:, ty * XW + tx : ty * XW + tx + FS],
                start=(t == 0), stop=(t == NPE1 - 1),
            )
        acc_a = work.tile([128, FS], BF16)
        av = acc_a[:]
        for i, t in enumerate(range(NPE1, 25)):
            ty, tx = divmod(t, 5)
            sv = x1p[:, ty * XW + tx : ty * XW + tx + FS]
            if i == 0:
                nc.vector.tensor_scalar(av, sv, w_dwp[:, t : t + 1], None, OP.mult)
            else:
                nc.vector.scalar_tensor_tensor(
                    av, sv, w_dwp[:, t : t + 1], av, OP.mult, OP.add
                )
        nc.vector.tensor_add(av, av, ps1[:])
        x2q = work.tile([128, 5 * W], BF16)
        nc.scalar.activation(
            x2q[:].rearrange("p (r c) -> p r c", c=W),
            acc_a[:].rearrange("p (r c) -> p r c", c=XW)[:, 0:5, 0:W],
            AF.Silu, bias=b_dwp,
        )

        # ---- comp px: 1x1 conv 64->64 (+ SiLU), split K: x1 from x12 rows,
        # x2 read directly from the packed x2p slabs (no writeback DMA)
        w_px1 = packa[0:32, 32:96]
        for g in range(4):
            ps = psA.tile([64, 5 * W], F32, tag="convps")
            nc.tensor.matmul(
                ps[:], w_px1,
                x12[0:32, (2 + 5 * g) * W : (7 + 5 * g) * W],
                start=True, stop=False,
            )
            base, cols = 64 * (g // 2), (246 if g % 2 == 0 else 310)
            nc.tensor.matmul(
                ps[:], packa[base : base + 64, cols : cols + 64],
                x2q[base : base + 64, :],
                start=False, stop=True,
            )
            nc.scalar.activation(
                enc_in[0:64, 5 * g * W : (5 * g + 5) * W], ps[:],
                AF.Silu, bias=b_px,
            )

        # ---- enc cv1: 1x1 conv 64->50 (+ SiLU, mask row rides K=65);
        # e1p pack DMAs issued as soon as their source rows are done
        for r0, nr in ((0, 8), (8, 8), (16, 4)):
            ps = psA.tile([50, 512], F32, tag="convps")
            nc.tensor.matmul(
                ps[:, : nr * W], w_ecv1,
                enc_in[0:65, r0 * W : (r0 + nr) * W],
                start=True, stop=True,
            )
            nc.scalar.activation(
                e1c[0:50, r0 * W : (r0 + nr) * W], ps[:, : nr * W],
                AF.Silu, bias=b_ecv1,
            )
            if r0 == 8:
                nc.sync.dma_start(
                    e1p3[0:50, 0:12, 2 : 2 + W], e1c3[0:50, 0:12, :]
                )
        nc.scalar.dma_start(
            e1p3[50:100, 0:12, 2 : 2 + W], e1c3[0:50, 8:20, :]
        )
        FS2 = 8 * XW                   # 544
        HF = FS2 // 2                  # 272
        ps2a = psD.tile([100, HF], F32, tag="dw2a")
        ps2b = psD.tile([100, HF], F32, tag="dw2b")
        for t in range(NPE2):
            ty, tx = divmod(t, 5)
            base = ty * XW + tx
            nc.tensor.matmul(
                ps2a[:], diag2[:, 100 * t : 100 * t + 100],
                e1p[0:100, base : base + HF],
                start=(t == 0), stop=(t == NPE2 - 1),
            )
        for t in range(NPE2):
            ty, tx = divmod(t, 5)
            base = ty * XW + tx
            nc.tensor.matmul(
                ps2b[:], diag2[:, 100 * t : 100 * t + 100],
                e1p[0:100, base + HF : base + FS2],
                start=(t == 0), stop=(t == NPE2 - 1),
            )
        acc2_a = work.tile([100, FS2], BF16)
        av2 = acc2_a[:]
        for i, t in enumerate(range(NPE2, 25)):
            ty, tx = divmod(t, 5)
            sv = e1p[:, ty * XW + tx : ty * XW + tx + FS2]
            if i == 0:
                nc.vector.tensor_scalar(av2, sv, w_edwp[:, t : t + 1], None, OP.mult)
            else:
                nc.vector.scalar_tensor_tensor(
                    av2, sv, w_edwp[:, t : t + 1], av2, OP.mult, OP.add
                )
        nc.vector.tensor_add(acc2_a[:, 0:HF], acc2_a[:, 0:HF], ps2a[:])
        nc.vector.tensor_add(acc2_a[:, HF:FS2], acc2_a[:, HF:FS2], ps2b[:])
        e2q = work.tile([100, 8 * W], BF16)
        nc.scalar.activation(
            e2q[:].rearrange("p (r c) -> p r c", c=W),
            acc2_a[:].rearrange("p (r c) -> p r c", c=XW)[:, 0:8, 0:W],
            AF.Silu, bias=b_edwp,
        )

        # xt load (needed by the out matmuls from ~mid-kernel only;
        # emitting it here avoids false semaphore deps in the conv front)
        for ch, eng in enumerate((nc.sync, nc.scalar)):
            eng.dma_start(
                xt[:, 2048 * ch : 2048 * (ch + 1)],
                xt_d[:, 2048 * ch : 2048 * (ch + 1)],
            )

        # ---- enc px (transposed output: M = 128 pixels per row-pair),
        # split K: e1 + bias row from e1c, e2 direct from e2p slabs
        w_epx_a = packa[0:51, 146:246]
        for t in range(8):
            g, lr = divmod(2 * t, 8)
            ps = psA.tile([128, 100], F32, tag="convps")
            nc.tensor.matmul(
                ps[:], e1c[0:51, (2 + 2 * t) * W : (4 + 2 * t) * W],
                w_epx_a, start=True, stop=False,
            )
            cols = 374 if g == 0 else 474
            nc.tensor.matmul(
                ps[:], e2q[0:100, lr * W : (lr + 2) * W],
                packa[0:100, cols : cols + 100],
                start=False, stop=True,
            )
            nc.scalar.activation(ET[:, 100 * t : 100 * t + 100], ps[:], AF.Silu)

        # ---- softmax over 25 taps per subposition (no max-subtraction)
        for s in range(4):
            nc.scalar.activation(exp3[:, s], ET3[:, :, s::4], AF.Exp)
            nc.vector.tensor_reduce(
                S[:, 8 * s : 8 * s + 8], exp3[:, s], mybir.AxisListType.X, OP.add
            )
        nc.vector.reciprocal(R[:], S[:])
        psD_cm.__exit__(None, None, None)
        psA_cm.__exit__(None, None, None)
        psO = ctx.enter_context(tc.tile_pool(name="psO", bufs=3, space="PSUM"))

        # normalized weights, s-major: wcat[p, 200s + 25t + k]
        # = exp3[p, s, t, k] * R[p, 8s+t]
        R3 = R[:].rearrange("p (s u) -> p s u", s=4)
        wcat4 = wcat[:].rearrange("p (s t k) -> p s t k", s=4, t=8)
        for t in range(8):
            nc.vector.tensor_tensor(
                wcat4[:, :, t],
                exp3[:, :, t],
                R3[:, :, t : t + 1].to_broadcast((128, 4, 25)),
                OP.mult,
            )

        # repl matmuls: per (jb, s) one [32, 200] output at psum partition
        # offset 32s (s-major pixel packing); then cast into dall2 with the
        # (tp, th, jb, k) column interleave the t-pair scatters consume.
        dall2v = dall[:, 0:800].rearrange(
            "p (tp th j k) -> p tp th j k", tp=4, th=2, j=4
        )
        for jb in range(4):
            ps = psB.tile([128, 200], F32, tag="repl")
            for s_ in range(4):
                nc.tensor.matmul(
                    ps[32 * s_ : 32 * s_ + 32, :],
                    repl[:, 128 * jb + 32 * s_ : 128 * jb + 32 * s_ + 32],
                    wcat[:, 200 * s_ : 200 * s_ + 200],
                    start=True, stop=True,
                    tile_position=(0, 32 * s_),
                )
            src3 = ps[:].rearrange("p (tp th k) -> p tp th k", tp=4, th=2)
            if jb % 2 == 0:
                nc.vector.tensor_copy(dall2v[:, :, :, jb], src3)
            else:
                nc.scalar.copy(dall2v[:, :, :, jb], src3)

        # scatters first (4 s-compacted calls, one t-pair each: 200 idx,
        # 1024 out) so the gpsimd queue never stalls behind downstream
        # DMAs; then per t-pair: one chunked DMA transpose -> per row-pair
        # 4 matmuls into a [128,512] PSUM bank -> 1 straight staging copy
        # (s-major pixel columns; the host unpermutes) -> out DMA.
        for tp in range(4):
            nc.gpsimd.local_scatter(
                b4t[:, 1024 * tp : 1024 * tp + 1024],
                dall[:, 200 * tp : 200 * tp + 200],
                sidx[:],
                channels=128, num_elems=1024, num_idxs=200,
            )
        stgs = []
        for tp in range(4):
            b4 = bpool.tile([128, 8, 128], BF16, tag="b4")
            (nc.sync if tp % 2 == 0 else nc.scalar).dma_start_transpose(
                b4[:], b4t[:, 1024 * tp : 1024 * tp + 1024]
            )
            for th in range(2):
                t = 2 * tp + th
                po = psO.tile([128, 512], F32, tag="out")
                for jb in range(4):
                    B = 4 * t + jb
                    nc.tensor.matmul(
                        po[:, 128 * jb : 128 * jb + 128],
                        xt[:, 128 * B : 128 * B + 128],
                        b4[0:120, 4 * th + jb, :],
                        start=True, stop=True,
                    )
                stg = spool.tile([128, 512], F32, tag="ostage")
                stgs.append(stg)
                if t % 2 == 0:
                    nc.vector.tensor_copy(stg[:], po[:])
                else:
                    nc.scalar.copy(stg[:], po[:])
                (nc.scalar if t % 2 == 0 else nc.sync).dma_start(
                    out3[:, 4 * t : 4 * t + 4, :],
                    stg[:].rearrange("c (r j) -> c r j", j=128),
                )

    nc.compile()
    return nc


_NC_CACHE = None


def _get_nc():
    global _NC_CACHE
    if _NC_CACHE is None:
        _NC_CACHE = build_kernel()
    return _NC_CACHE


def kernel(**inputs) -> np.ndarray:
    X = np.asarray(inputs["X"], np.float32)
    consts = _host_consts(
        {k: np.asarray(v, np.float32) for k, v in inputs.items() if k != "X"}
    )
    in_maps = []
    for core in range(NCORES):
        xs, mrow, emask, xt = _host_shard(X, core)
        m = dict(consts)
        m["x"] = xs
        m["mrow"] = mrow
        m["emask"] = emask
        m["xt"] = xt
        in_maps.append(m)

    nc = _get_nc()
    res = run_bass_kernel_spmd(nc, in_maps, core_ids=list(range(NCORES)))
    out = np.zeros((2, C, 128, 128), np.float32)
    for core in range(NCORES):
        b, ri = divmod(core, 4)
        # device rows are [t, jb]; cols are [dy, dx, yl, xl] (s-major)
        v = res.results[core]["out"].reshape(C, 8, 4, 2, 2, 2, 16)
        v = v.transpose(0, 1, 5, 3, 2, 6, 4).reshape(C, 32, 128)
        out[b, :, 32 * ri : 32 * ri + 32, :] = v
    return out


if __name__ == "__main__":
    print("smoke build only")
    build_kernel()
    print("build ok")


# revision 30
# speedup vs baseline: 1.0806x; 1.0333x over previous
"""CARAFE + MSGConv Trainium2 kernel (8 NeuronCores, spatial x batch sharding).

out[c, i, j] = sum_{p,q} W[5p+q, i, j] * Xpad[c, i//2 + p - 2, j//2 + q - 2]
 (CARAFE taps live at source resolution; identical for both subpixel parities).

Per core: one batch element (core//4) and a 16-source-row block (core%4).

v2 changes vs baseline:
 - dw tap loops split Vector (STT) || TensorEngine (diagonal-weight matmuls
   accumulating in PSUM), combined with one tensor_add then Silu.
 - b4 transpose moved from PE (transpose + PSUM->SBUF copy) to DMA xbar
   transpose (SBUF->SBUF, [128,128] bf16 blocks).
 - repl matmuls merged 2 row-pairs per call (16x N=200).
 - out matmuls write 4 jb blocks into one [128,512] PSUM bank; single
   staging copy per row-pair.
 - wcat softmax scaling on vector tensor_scalar (2x mode).
 - input DMA ordering: front-critical tensors first, xt/back-end consts later.
"""

import sys

sys.path.insert(0, "/opt/trn_rl_repo")

from contextlib import ExitStack

import ml_dtypes
import numpy as np

import concourse.bass as bass
import concourse.tile as tile
from concourse import bacc, library_config, mybir
from concourse.bass_utils import run_bass_kernel_spmd

BF16 = mybir.dt.bfloat16
F32 = mybir.dt.float32
I16 = mybir.dt.int16
AF = mybir.ActivationFunctionType
OP = mybir.AluOpType
nbf = ml_dtypes.bfloat16

C = 128
H = W = 64
NCORES = 8
XR = 24          # X shard rows (16 + 4 halo each side)
XW = 68          # padded width for dw slabs only
NEG = -30.0      # additive pre-activation mask; SiLU(-30) ~= -2.8e-12

# dw tap split: taps [0, NPE) on the TensorEngine, [NPE, 25) on Vector
NPE1 = 17        # dw1 PE taps
NPE2 = 16        # dw2 PE taps


# ======================================================================
# host-side parameter prep
# ======================================================================

def _fold_1x1(w, s):
    return (w[:, :, 0, 0] * s[:, None]).T.copy()


def _dw_taps(w, s, k):
    ch = w.shape[0]
    out = np.zeros((ch, 25), np.float32)
    off = (5 - k) // 2
    for ty in range(k):
        for tx in range(k):
            out[:, 5 * (ty + off) + (tx + off)] = w[:, 0, ty, tx] * s
    return out


def _host_consts(inputs):
    d = {}
    w_cv1 = _fold_1x1(inputs["comp_cv1_w"], inputs["comp_cv1_s"])
    b_cv1 = inputs["comp_cv1_b"].reshape(32, 1)
    w3 = _dw_taps(inputs["comp_dw3_w"], inputs["comp_dw3_s"], 3)
    w5 = _dw_taps(inputs["comp_dw5_w"], inputs["comp_dw5_s"], 5)
    w_dwp = np.tile(np.concatenate([w3, w5], 0), (4, 1))
    b_dwp = np.tile(
        np.concatenate([inputs["comp_dw3_b"], inputs["comp_dw5_b"]]), 4
    ).reshape(128, 1)
    w_px = _fold_1x1(inputs["comp_px_w"], inputs["comp_px_s"])
    b_px = inputs["comp_px_b"].reshape(64, 1)
    we = _fold_1x1(inputs["enc_cv1_w"], inputs["enc_cv1_s"])
    w_ecv1 = np.concatenate([we, np.ones((1, 50), np.float32)], 0)
    b_ecv1 = inputs["enc_cv1_b"].reshape(50, 1)
    e3 = _dw_taps(inputs["enc_dw3_w"], inputs["enc_dw3_s"], 3)
    e5 = _dw_taps(inputs["enc_dw5_w"], inputs["enc_dw5_s"], 5)
    w_edwp = np.tile(np.concatenate([e3, e5], 0), (2, 1))
    b_edwp = np.tile(
        np.concatenate([inputs["enc_dw3_b"], inputs["enc_dw5_b"]]), 2
    ).reshape(100, 1)
    wpx = _fold_1x1(inputs["enc_px_w"], inputs["enc_px_s"])
    w_epx = np.concatenate([wpx, inputs["enc_px_b"].reshape(1, 100)], 0)

    # packA bf16 [128, 510]: w_cv1 | w_px | w_ecv1 | w_epx_a | px2 masked
    # halves (even/odd dw1 groups) | epx_b masked halves (lo/hi e2p group)
    pa = np.zeros((128, 574), np.float32)
    pa[0:128, 0:32] = w_cv1
    pa[0:64, 32:96] = w_px
    pa[0:65, 96:146] = w_ecv1
    pa[0:50, 146:246] = w_epx[0:50]
    pa[50:51, 146:246] = w_epx[100:101]
    pa[0:32, 246:310] = w_px[32:64]      # g even (lhsT half [0:64] base 0/64)
    pa[64:96, 246:310] = w_px[32:64]
    pa[32:64, 310:374] = w_px[32:64]     # g odd
    pa[96:128, 310:374] = w_px[32:64]
    pa[0:50, 374:474] = w_epx[50:100]    # e2p group 0 (t < 4)
    pa[50:100, 474:574] = w_epx[50:100]  # e2p group 1 (t >= 4)
    d["packa"] = pa.astype(nbf)
    # packB f32 [128, 55]
    pb = np.zeros((128, 55), np.float32)
    pb[:, 0:25] = w_dwp
    pb[:, 25:26] = b_dwp
    pb[0:100, 26:51] = w_edwp
    pb[0:100, 51:52] = b_edwp
    pb[0:32, 52:53] = b_cv1
    pb[0:64, 53:54] = b_px
    pb[0:50, 54:55] = b_ecv1
    d["packb"] = pb

    # diagonal lhsT for the PE dw taps
    dg1 = np.zeros((128, NPE1 * 128), np.float32)
    for t in range(NPE1):
        dg1[np.arange(128), 128 * t + np.arange(128)] = w_dwp[:, t]
    d["diag1"] = dg1.astype(nbf)
    dg2 = np.zeros((100, NPE2 * 100), np.float32)
    for t in range(NPE2):
        dg2[np.arange(100), 100 * t + np.arange(100)] = w_edwp[:, t]
    d["diag2"] = dg2.astype(nbf)

    d["ones1"] = np.ones((1, 32), nbf)

    # replS [128, 4*128]: lhsT blocks per (jb, s); output pixel partition
    # within a jb block is s-major: m = 16*yl + xl, placed at psum
    # partitions 32s+m by the matmul's out slice.
    rp = np.zeros((128, 512), np.float32)
    for jb in range(4):
        for s_ in range(4):
            for yl in range(2):
                for xl in range(16):
                    rp[64 * yl + 16 * jb + xl,
                       128 * jb + 32 * s_ + 16 * yl + xl] = 1.0
    d["repl"] = rp.astype(nbf)

    # sidx [128, 200] int16, shared by all four t-pair scatter calls:
    # partition = s-major pixel-in-block (32s + 16yl + xl), data col
    # (th, jb, k) -> target slot 512*th + 128*jb + 20u + v (uv pitch 20,
    # cols 120..127 of each 128 block are pad for the xbar transpose).
    si = np.full((128, 200), -1, np.int16)
    for part in range(128):
        s_, m = divmod(part, 32)
        yl, xl = divmod(m, 16)
        for th in range(2):
            for jb in range(4):
                for k in range(25):
                    p, q = divmod(k, 5)
                    if not (0 <= 16 * jb + xl + q - 2 < 64):
                        continue
                    si[part, 100 * th + 25 * jb + k] = (
                        512 * th + 128 * jb + 20 * (yl + p) + (xl + q)
                    )
    d["sidx"] = si
    return d


def _host_shard(X, core):
    b, ri = divmod(core, 4)
    r0 = 16 * ri - 4
    xs = np.zeros((C, XR, W), np.float32)
    lo, hi = max(0, r0), min(H, r0 + XR)
    xs[:, lo - r0 : hi - r0, :] = X[b, :, lo:hi, :]
    mrow = np.zeros((1, XR, W), np.float32)
    for r in range(XR):
        if not (0 <= r0 + r < H):
            mrow[0, r, :] = NEG
    emask = np.zeros((1, 20, W), np.float32)
    for r in range(20):
        if not (0 <= (16 * ri - 2) + r < H):
            emask[0, r, :] = NEG
    xsb = xs.astype(nbf)
    # pre-transposed X slabs, one [120, 128] per block (column-padded)
    xsp = np.zeros((C, XR, XW), nbf)
    xsp[:, :, 2 : 2 + W] = xsb
    xt = np.zeros((120, 32 * 128), nbf)
    for B in range(32):
        t, jb = divmod(B, 4)
        slab = xsp[:, 2 * t + 2 : 2 * t + 8, 16 * jb : 16 * jb + 20]
        xt[:, 128 * B : 128 * B + 128] = slab.reshape(C, 120).T
    return (
        xsb.reshape(C, XR * W),
        mrow.reshape(1, XR * W).astype(nbf),
        emask.reshape(1, 20 * W).astype(nbf),
        xt,
    )


# ======================================================================
# device kernel
# ======================================================================

def build_kernel():
    nc = bacc.Bacc(
        "TRN2",
        target_bir_lowering=False,
        debug=False,
        enable_asserts=False,
        num_devices=NCORES,
    )

    def din(name, shape, dt):
        return nc.dram_tensor(name, list(shape), dt, kind="ExternalInput").ap()

    x_d = din("x", (128, XR * W), BF16)
    xt_d = din("xt", (120, 32 * 128), BF16)
    mrow_d = din("mrow", (1, XR * W), BF16)
    emask_d = din("emask", (1, 20 * W), BF16)
    ones1_d = din("ones1", (1, 32), BF16)
    packa_d = din("packa", (128, 574), BF16)
    packb_d = din("packb", (128, 55), F32)
    diag1_d = din("diag1", (128, NPE1 * 128), BF16)
    diag2_d = din("diag2", (100, NPE2 * 100), BF16)
    repl_d = din("repl", (128, 512), BF16)
    sidx_d = din("sidx", (128, 200), I16)
    out_d = nc.dram_tensor("out", [128, 32 * 128], F32, kind="ExternalOutput").ap()
    out3 = out_d.rearrange("c (r j) -> c r j", j=128)

    with tile.TileContext(nc) as tc, ExitStack() as ctx:
        cpool = ctx.enter_context(tc.tile_pool(name="consts", bufs=1))
        work = ctx.enter_context(tc.tile_pool(name="work", bufs=1))
        psB = ctx.enter_context(tc.tile_pool(name="psB", bufs=2, space="PSUM"))
        spool = ctx.enter_context(tc.tile_pool(name="stage", bufs=4))
        bpool = ctx.enter_context(tc.tile_pool(name="b4s", bufs=6))
        psA_cm = tc.tile_pool(name="psA", bufs=2, space="PSUM")
        psA = psA_cm.__enter__()
        psD_cm = tc.tile_pool(name="psD", bufs=1, space="PSUM")
        psD = psD_cm.__enter__()

        nc.gpsimd.load_library(library_config.local_scatter)

        def cload(ap_d, shape, dt, eng=None):
            t = cpool.tile(list(shape), dt, tag=ap_d.tensor.name)
            (eng or nc.sync).dma_start(t[:], ap_d)
            return t

        # front-critical loads first, in queue-program order per engine
        xb = cpool.tile([128, XR * W], BF16, tag="x")
        for ch, eng in enumerate((nc.sync, nc.scalar, nc.gpsimd)):
            eng.dma_start(
                xb[:, 8 * W * ch : 8 * W * (ch + 1)],
                x_d[:, 8 * W * ch : 8 * W * (ch + 1)],
            )
        packa = cload(packa_d, (128, 574), BF16)
        mrow = cload(mrow_d, (1, XR * W), BF16, eng=nc.scalar)
        packb = cload(packb_d, (128, 55), F32, eng=nc.scalar)
        ones1 = cload(ones1_d, (1, 32), BF16, eng=nc.scalar)
        diag1 = cpool.tile([128, NPE1 * 128], BF16, tag="diag1")
        diag2 = cpool.tile([100, NPE2 * 100], BF16, tag="diag2")
        # back-end consts (needed later) on the gpsimd software queue
        repl = cload(repl_d, (128, 512), BF16, eng=nc.gpsimd)
        sidx = cload(sidx_d, (128, 200), I16, eng=nc.gpsimd)
        xt = cpool.tile([120, 32 * 128], BF16, tag="xt")

        w_cv1 = packa[0:128, 0:32]
        w_px = packa[0:64, 32:96]
        w_ecv1 = packa[0:65, 96:146]
        w_epx = packa[0:101, 146:246]
        w_dwp = packb[0:128, 0:25]
        b_dwp = packb[0:128, 25:26]
        w_edwp = packb[0:100, 26:51]
        b_edwp = packb[0:100, 51:52]
        b_cv1 = packb[0:32, 52:53]
        b_px = packb[0:64, 53:54]
        b_ecv1 = packb[0:50, 54:55]

        # warmup: trigger the local_scatter ucode library load early so it
        # overlaps the conv front instead of stalling the first real scatter
        warm = work.tile([16, 16], BF16)
        nc.gpsimd.local_scatter(
            warm[:], packa[0:16, 0:2], sidx[:][0:16, 0:2],
            channels=16, num_elems=16, num_idxs=2,
        )

        # persistent working tensors (all 64-wide / contiguous)
        x12 = work.tile([64, XR * W], BF16)        # x1 (0:32) + x2 (32:64)
        enc_in = work.tile([65, 20 * W], BF16)     # px out + mask row
        e1c = work.tile([51, 20 * W], BF16)        # enc cv1 out + ones row
        x1p = work.tile([128, 9 * XW + 8], BF16)   # packed x1 (68-pitch)
        e1p = work.tile([100, 12 * XW + 8], BF16)  # packed enc x1 (68-pitch)
        ET = work.tile([128, 800], BF16)
        expv = work.tile([128, 800], BF16)         # [s][t][k]
        S = work.tile([128, 32], F32)
        R = work.tile([128, 32], F32)
        wcat = work.tile([128, 800], BF16)         # [t][s][k]
        dall = work.tile([128, 3200], BF16)
        b4t = work.tile([128, 8 * 512], BF16)      # per t: [4jb x 128]

        xb3 = xb[:].rearrange("p (r c) -> p r c", c=W)
        x12_3 = x12[:].rearrange("p (r c) -> p r c", c=W)
        e1c3 = e1c[:].rearrange("p (r c) -> p r c", c=W)
        x1p3 = x1p[:, 0 : 9 * XW].rearrange("p (r c) -> p r c", c=XW)
        e1p3 = e1p[:, 0 : 12 * XW].rearrange("p (r c) -> p r c", c=XW)
        ET3 = ET[:].rearrange("p (t e) -> p t e", e=100)
        exp3 = expv[:].rearrange("p (s t k) -> p s t k", s=4, t=8)

        # zero only the dw-slab pad columns (cols 0:2 and 66:68)
        nc.vector.memset(x1p[:, 9 * XW : 9 * XW + 8], 0.0)
        nc.vector.memset(e1p[:, 12 * XW : 12 * XW + 8], 0.0)
        nc.vector.memset(x1p3[:, :, 0:2], 0.0)
        nc.vector.memset(x1p3[:, :, 66:68], 0.0)
        nc.vector.memset(e1p3[:, :, 0:2], 0.0)
        nc.vector.memset(e1p3[:, :, 66:68], 0.0)
        nc.vector.memset(e1c[:], 1.0)

        # ---- comp cv1: 1x1 conv 128->32 (+ SiLU + out-of-image row mask)
        for ch in range(3):
            ps = psA.tile([32, 512], F32, tag="convps")
            nc.tensor.matmul(
                ps[:], w_cv1, xb[:, 512 * ch : 512 * (ch + 1)],
                start=True, stop=False,
            )
            nc.tensor.matmul(
                ps[:], ones1[:], mrow[:, 512 * ch : 512 * (ch + 1)],
                start=False, stop=True,
            )
            nc.scalar.activation(
                x12[0:32, 512 * ch : 512 * (ch + 1)], ps[:],
                AF.Silu, bias=b_cv1,
            )

        # dw diag weights + enc mask: needed from the dw stages onward;
        # emitted here so their transfers don't clog the early queues
        nc.sync.dma_start(diag1[:], diag1_d)
        nc.scalar.dma_start(diag2[:], diag2_d)
        nc.gpsimd.dma_start(enc_in[64:65, :], emask_d)

        # ---- comp dw3/dw5 (unified 5x5 taps, rows packed 4x32)
        # taps [0, NPE1) on the TensorEngine (diag lhsT, PSUM accumulate),
        # taps [NPE1, 25) on Vector (STT chain); combined + Silu.
        for g, eng in enumerate((nc.sync, nc.scalar, nc.gpsimd, nc.sync)):
            eng.dma_start(
                x1p3[32 * g : 32 * g + 32, 0:9, 2 : 2 + W],
                x12_3[0:32, 5 * g : 5 * g + 9, :],
            )
        FS = 5 * XW                    # 340
        ps1 = psD.tile([128, FS], F32, tag="dw1")
        for t in range(NPE1):
            ty, tx = divmod(t, 5)
            nc.tensor.matmul(
                ps1[:], diag1[:, 128 * t : 128 * t + 128],
                x1p[:, ty * XW + tx : ty * XW + tx + FS],
                start=(t == 0), stop=(t == NPE1 - 1),
            )
        acc_a = work.tile([128, FS], BF16)
        av = acc_a[:]
        for i, t in enumerate(range(NPE1, 25)):
            ty, tx = divmod(t, 5)
            sv = x1p[:, ty * XW + tx : ty * XW + tx + FS]
            if i == 0:
                nc.vector.tensor_scalar(av, sv, w_dwp[:, t : t + 1], None, OP.mult)
            else:
                nc.vector.scalar_tensor_tensor(
                    av, sv, w_dwp[:, t : t + 1], av, OP.mult, OP.add
                )
        nc.vector.tensor_add(av, av, ps1[:])
        x2q = work.tile([128, 5 * W], BF16)
        nc.scalar.activation(
            x2q[:].rearrange("p (r c) -> p r c", c=W),
            acc_a[:].rearrange("p (r c) -> p r c", c=XW)[:, 0:5, 0:W],
            AF.Silu, bias=b_dwp,
        )

        # ---- comp px: 1x1 conv 64->64 (+ SiLU), split K: x1 from x12 rows,
        # x2 read directly from the packed x2p slabs (no writeback DMA)
        w_px1 = packa[0:32, 32:96]
        for g in range(4):
            ps = psA.tile([64, 5 * W], F32, tag="convps")
            nc.tensor.matmul(
                ps[:], w_px1,
                x12[0:32, (2 + 5 * g) * W : (7 + 5 * g) * W],
                start=True, stop=False,
            )
            base, cols = 64 * (g // 2), (246 if g % 2 == 0 else 310)
            nc.tensor.matmul(
                ps[:], packa[base : base + 64, cols : cols + 64],
                x2q[base : base + 64, :],
                start=False, stop=True,
            )
            nc.scalar.activation(
                enc_in[0:64, 5 * g * W : (5 * g + 5) * W], ps[:],
                AF.Silu, bias=b_px,
            )

        # ---- enc cv1: 1x1 conv 64->50 (+ SiLU, mask row rides K=65);
        # e1p pack DMAs issued as soon as their source rows are done
        for r0, nr in ((0, 8), (8, 8), (16, 4)):
            ps = psA.tile([50, 512], F32, tag="convps")
            nc.tensor.matmul(
                ps[:, : nr * W], w_ecv1,
                enc_in[0:65, r0 * W : (r0 + nr) * W],
                start=True, stop=True,
            )
            nc.scalar.activation(
                e1c[0:50, r0 * W : (r0 + nr) * W], ps[:, : nr * W],
                AF.Silu, bias=b_ecv1,
            )
            if r0 == 0:
                nc.sync.dma_start(
                    e1p3[0:50, 0:8, 2 : 2 + W], e1c3[0:50, 0:8, :]
                )
            if r0 == 8:
                nc.scalar.dma_start(
                    e1p3[0:50, 8:12, 2 : 2 + W], e1c3[0:50, 8:12, :]
                )
                nc.gpsimd.dma_start(
                    e1p3[50:100, 0:8, 2 : 2 + W], e1c3[0:50, 8:16, :]
                )
        nc.sync.dma_start(
            e1p3[50:100, 8:12, 2 : 2 + W], e1c3[0:50, 16:20, :]
        )
        FS2 = 8 * XW                   # 544
        HF = FS2 // 2                  # 272
        ps2a = psD.tile([100, HF], F32, tag="dw2a")
        ps2b = psD.tile([100, HF], F32, tag="dw2b")
        for t in range(NPE2):
            ty, tx = divmod(t, 5)
            base = ty * XW + tx
            nc.tensor.matmul(
                ps2a[:], diag2[:, 100 * t : 100 * t + 100],
                e1p[0:100, base : base + HF],
                start=(t == 0), stop=(t == NPE2 - 1),
            )
        for t in range(NPE2):
            ty, tx = divmod(t, 5)
            base = ty * XW + tx
            nc.tensor.matmul(
                ps2b[:], diag2[:, 100 * t : 100 * t + 100],
                e1p[0:100, base + HF : base + FS2],
                start=(t == 0), stop=(t == NPE2 - 1),
            )
        acc2_a = work.tile([100, FS2], BF16)
        av2 = acc2_a[:]
        for i, t in enumerate(range(NPE2, 25)):
            ty, tx = divmod(t, 5)
            sv = e1p[:, ty * XW + tx : ty * XW + tx + FS2]
            if i == 0:
                nc.vector.tensor_scalar(av2, sv, w_edwp[:, t : t + 1], None, OP.mult)
            else:
                nc.vector.scalar_tensor_tensor(
                    av2, sv, w_edwp[:, t : t + 1], av2, OP.mult, OP.add
                )
        nc.vector.tensor_add(acc2_a[:, 0:HF], acc2_a[:, 0:HF], ps2a[:])
        nc.vector.tensor_add(acc2_a[:, HF:FS2], acc2_a[:, HF:FS2], ps2b[:])
        e2q = work.tile([100, 8 * W], BF16)
        nc.scalar.activation(
            e2q[:].rearrange("p (r c) -> p r c", c=W),
            acc2_a[:].rearrange("p (r c) -> p r c", c=XW)[:, 0:8, 0:W],
            AF.Silu, bias=b_edwp,
        )

        # xt load (needed by the out matmuls from ~mid-kernel only;
        # emitting it here avoids false semaphore deps in the conv front)
        for ch, eng in enumerate((nc.sync, nc.scalar)):
            eng.dma_start(
                xt[:, 2048 * ch : 2048 * (ch + 1)],
                xt_d[:, 2048 * ch : 2048 * (ch + 1)],
            )

        # ---- enc px (transposed output: M = 128 pixels per row-pair),
        # split K: e1 + bias row from e1c, e2 direct from e2p slabs
        w_epx_a = packa[0:51, 146:246]
        for t in range(8):
            g, lr = divmod(2 * t, 8)
            ps = psA.tile([128, 100], F32, tag="convps")
            nc.tensor.matmul(
                ps[:], e1c[0:51, (2 + 2 * t) * W : (4 + 2 * t) * W],
                w_epx_a, start=True, stop=False,
            )
            cols = 374 if g == 0 else 474
            nc.tensor.matmul(
                ps[:], e2q[0:100, lr * W : (lr + 2) * W],
                packa[0:100, cols : cols + 100],
                start=False, stop=True,
            )
            nc.scalar.activation(ET[:, 100 * t : 100 * t + 100], ps[:], AF.Silu)

        # ---- softmax over 25 taps per subposition (no max-subtraction)
        for s in range(4):
            nc.scalar.activation(exp3[:, s], ET3[:, :, s::4], AF.Exp)
            nc.vector.tensor_reduce(
                S[:, 8 * s : 8 * s + 8], exp3[:, s], mybir.AxisListType.X, OP.add
            )
        nc.vector.reciprocal(R[:], S[:])
        psD_cm.__exit__(None, None, None)
        psA_cm.__exit__(None, None, None)
        psO = ctx.enter_context(tc.tile_pool(name="psO", bufs=3, space="PSUM"))

        # normalized weights, s-major: wcat[p, 200s + 25t + k]
        # = exp3[p, s, t, k] * R[p, 8s+t]
        R3 = R[:].rearrange("p (s u) -> p s u", s=4)
        wcat4 = wcat[:].rearrange("p (s t k) -> p s t k", s=4, t=8)
        for t in range(8):
            nc.vector.tensor_tensor(
                wcat4[:, :, t],
                exp3[:, :, t],
                R3[:, :, t : t + 1].to_broadcast((128, 4, 25)),
                OP.mult,
            )

        # repl matmuls: per (jb, s) one [32, 200] output at psum partition
        # offset 32s (s-major pixel packing); then cast into dall2 with the
        # (tp, th, jb, k) column interleave the t-pair scatters consume.
        dall2v = dall[:, 0:800].rearrange(
            "p (tp th j k) -> p tp th j k", tp=4, th=2, j=4
        )
        for jb in range(4):
            ps = psB.tile([128, 200], F32, tag="repl")
            for s_ in range(4):
                nc.tensor.matmul(
                    ps[32 * s_ : 32 * s_ + 32, :],
                    repl[:, 128 * jb + 32 * s_ : 128 * jb + 32 * s_ + 32],
                    wcat[:, 200 * s_ : 200 * s_ + 200],
                    start=True, stop=True,
                    tile_position=(0, 32 * s_),
                )
            src3 = ps[:].rearrange("p (tp th k) -> p tp th k", tp=4, th=2)
            if jb % 2 == 0:
                nc.vector.tensor_copy(dall2v[:, :, :, jb], src3)
            else:
                nc.scalar.copy(dall2v[:, :, :, jb], src3)

        # scatters first (4 s-compacted calls, one t-pair each: 200 idx,
        # 1024 out) so the gpsimd queue never stalls behind downstream
        # DMAs; then per t-pair: one chunked DMA transpose -> per row-pair
        # 4 matmuls into a [128,512] PSUM bank -> 1 straight staging copy
        # (s-major pixel columns; the host unpermutes) -> out DMA.
        for tp in range(4):
            nc.gpsimd.local_scatter(
                b4t[:, 1024 * tp : 1024 * tp + 1024],
                dall[:, 200 * tp : 200 * tp + 200],
                sidx[:],
                channels=128, num_elems=1024, num_idxs=200,
            )
        stgs = []
        for tp in range(4):
            b4 = bpool.tile([128, 8, 128], BF16, tag="b4")
            (nc.sync if tp % 2 == 0 else nc.scalar).dma_start_transpose(
                b4[:], b4t[:, 1024 * tp : 1024 * tp + 1024]
            )
            for th in range(2):
                t = 2 * tp + th
                po = psO.tile([128, 512], F32, tag="out")
                for jb in range(4):
                    B = 4 * t + jb
                    nc.tensor.matmul(
                        po[:, 128 * jb : 128 * jb + 128],
                        xt[:, 128 * B : 128 * B + 128],
                        b4[0:120, 4 * th + jb, :],
                        start=True, stop=True,
                    )
                stg = spool.tile([128, 512], F32, tag="ostage")
                stgs.append(stg)
                if t % 2 == 0:
                    nc.vector.tensor_copy(stg[:], po[:])
                else:
                    nc.scalar.copy(stg[:], po[:])
                (nc.scalar if t % 2 == 0 else nc.sync).dma_start(
                    out3[:, 4 * t : 4 * t + 4, :],
                    stg[:].rearrange("c (r j) -> c r j", j=128),
                )

    nc.compile()
    return nc


_NC_CACHE = None


def _get_nc():
    global _NC_CACHE
    if _NC_CACHE is None:
        _NC_CACHE = build_kernel()
    return _NC_CACHE


def kernel(**inputs) -> np.ndarray:
    X = np.asarray(inputs["X"], np.float32)
    consts = _host_consts(
        {k: np.asarray(v, np.float32) for k, v in inputs.items() if k != "X"}
    )
    in_maps = []
    for core in range(NCORES):
        xs, mrow, emask, xt = _host_shard(X, core)
        m = dict(consts)
        m["x"] = xs
        m["mrow"] = mrow
        m["emask"] = emask
        m["xt"] = xt
        in_maps.append(m)

    nc = _get_nc()
    res = run_bass_kernel_spmd(nc, in_maps, core_ids=list(range(NCORES)))
    out = np.zeros((2, C, 128, 128), np.float32)
    for core in range(NCORES):
        b, ri = divmod(core, 4)
        # device rows are [t, jb]; cols are [dy, dx, yl, xl] (s-major)
        v = res.results[core]["out"].reshape(C, 8, 4, 2, 2, 2, 16)
        v = v.transpose(0, 1, 5, 3, 2, 6, 4).reshape(C, 32, 128)
        out[b, :, 32 * ri : 32 * ri + 32, :] = v
    return out


if __name__ == "__main__":
    print("smoke build only")
    build_kernel()
    print("build ok")
